# revision 1
# baseline (speedup 1.0000x reference)
"""Trainium2 Bass kernel for nn_MoE_89498528514729 (moe_routing).

Expert-parallel sparse MoE across 8 NeuronCores:
  - every core gets the full x; routed experts are sharded 2-per-core
  - gate (sigmoid scores + group-limited top-4) computed on device in fp32
  - dispatch tables built on device (tensor_tensor_scan + local_scatter)
  - per-expert token gather via dma_gather (transposed, fp16)
  - SwiGLU expert FFN in fp16 (fp32 PSUM accumulation)
  - weighted outputs scatter-added into a token-ordered partial-sum buffer
  - ReduceScatter combines partials across cores; each core finishes its
    256-token slice by adding the (token-sliced) shared expert output
Host side only shards/transposes/casts inputs and concatenates outputs.
"""

import numpy as np

import concourse.bass as bass
import concourse.mybir as mybir
import concourse.tile as tile
from concourse import bacc
from concourse.masks import make_identity

P = 128
T = 2048
D = 1024
II = 512
E = 16
EL = 2          # experts per core
NCORES = 8
TS = T // NCORES  # tokens per core output slice
C = 640         # per-expert token capacity (multiple of 128; actual max count 553)
CW = C // 16    # wrapped index width
BIG = 1.0e30
USE_SILU = True  # sim lacks Silu; set False for CoreSim runs
ABLATE = None  # None | 'experts' | 'gate'  (timeline bisection only)

f32 = mybir.dt.float32
f16 = mybir.dt.float16
i16 = mybir.dt.int16
i32 = mybir.dt.int32
Alu = mybir.AluOpType
Act = mybir.ActivationFunctionType


def build_kernel(n_cores: int = NCORES):
    nc = bacc.Bacc("TRN2", target_bir_lowering=False, debug=False, num_devices=n_cores)

    # ---------------- external tensors ----------------
    x16 = nc.dram_tensor("x16", [T, D], f16, kind="ExternalInput")
    xT32 = nc.dram_tensor("xT32", [D, T], f32, kind="ExternalInput")
    gwT = nc.dram_tensor("gwT", [D, E], f32, kind="ExternalInput")
    gb = nc.dram_tensor("gb", [1, E], f32, kind="ExternalInput")
    esel = nc.dram_tensor("esel", [EL, E], f32, kind="ExternalInput")
    w1T = nc.dram_tensor("w1T", [EL, D, II], f16, kind="ExternalInput")
    w3T = nc.dram_tensor("w3T", [EL, D, II], f16, kind="ExternalInput")
    w2T = nc.dram_tensor("w2T", [EL, II, D], f16, kind="ExternalInput")
    ws1T = nc.dram_tensor("ws1T", [D, II], f16, kind="ExternalInput")
    ws3T = nc.dram_tensor("ws3T", [D, II], f16, kind="ExternalInput")
    ws2T = nc.dram_tensor("ws2T", [II, D], f16, kind="ExternalInput")
    xTs = nc.dram_tensor("xTs", [D, TS], f16, kind="ExternalInput")
    out = nc.dram_tensor("out", [TS, D], f32, kind="ExternalOutput")

    with tile.TileContext(nc) as tc:
        _body(nc, tc, n_cores, locals())
    nc.compile()
    return nc


def _body(nc, tc, n_cores, t_):
    x16, xT32, gwT, gb, esel = t_["x16"], t_["xT32"], t_["gwT"], t_["gb"], t_["esel"]
    w1T, w3T, w2T = t_["w1T"], t_["w3T"], t_["w2T"]
    ws1T, ws3T, ws2T, xTs, out = t_["ws1T"], t_["ws3T"], t_["ws2T"], t_["xTs"], t_["out"]

    import contextlib
    ctx = contextlib.ExitStack()
    with ctx:
        const = ctx.enter_context(tc.tile_pool(name="const", bufs=1))
        wpool = ctx.enter_context(tc.tile_pool(name="wpool", bufs=1))
        gpool = ctx.enter_context(tc.tile_pool(name="gpool", bufs=1))
        spool = ctx.enter_context(tc.tile_pool(name="spool", bufs=2))
        cdp = ctx.enter_context(tc.tile_pool(name="cdp", bufs=1))
        xpool = ctx.enter_context(tc.tile_pool(name="xpool", bufs=1))
        hpool = ctx.enter_context(tc.tile_pool(name="hpool", bufs=1))
        ypool = ctx.enter_context(tc.tile_pool(name="ypool", bufs=2))
        ps_t = ctx.enter_context(tc.tile_pool(name="ps_t", bufs=2, space="PSUM"))
        ps_h = ctx.enter_context(tc.tile_pool(name="ps_h", bufs=2, space="PSUM"))
        ps_y = ctx.enter_context(tc.tile_pool(name="ps_y", bufs=2, space="PSUM"))
        dram = ctx.enter_context(tc.tile_pool(name="dram", bufs=1, space="DRAM"))

        # ---------------- DRAM internals ----------------
        comb_dram = dram.tile([T, 64], f32)
        msk_dram = dram.tile([4, T], f32)
        gth_dram = dram.tile([EL, 16, CW], i16)
        y_dram = dram.tile([T, D], f16)
        rs_out = dram.tile([TS, D], f16)

        # ---------------- constants & weight loads ----------------
        ident = const.tile([P, P], f32)
        make_identity(nc, ident[:])
        bias_sb = const.tile([P, E], f32)
        nc.sync.dma_start(bias_sb[:], gb[0:1, :].to_broadcast([P, E]))
        esel_sb = const.tile([P, EL, E], f32)
        nc.sync.dma_start(esel_sb[:], esel[None, :, :].to_broadcast([P, EL, E]))
        gwT_sb = const.tile([P, D // P, E], f32)
        nc.sync.dma_start(gwT_sb[:], gwT.ap().rearrange("(ko p) e -> p ko e", p=P))

        ws1_sb = wpool.tile([P, D // P, II], f16, tag="ws1")
        nc.scalar.dma_start(ws1_sb[:], ws1T.ap().rearrange("(ko p) i -> p ko i", p=P))
        ws3_sb = wpool.tile([P, D // P, II], f16, tag="ws3")
        nc.scalar.dma_start(ws3_sb[:], ws3T.ap().rearrange("(ko p) i -> p ko i", p=P))
        ws2_sb = wpool.tile([P, II // P, D], f16, tag="ws2")
        nc.scalar.dma_start(ws2_sb[:], ws2T.ap().rearrange("(ko p) d -> p ko d", p=P))
        xTs_sb = wpool.tile([P, D // P, TS], f16, tag="xTs")
        nc.scalar.dma_start(xTs_sb[:], xTs.ap().rearrange("(ko p) t -> p ko t", p=P))

        if ABLATE != 'gate':
            # ---------------- gate: scoresT = sigmoid(gw @ xT) ----------------
            scoresT = gpool.tile([E, T], f32, tag="slab8k")
            GC = 256
            for j in range(T // GC):
                xg = spool.tile([P, D // P, GC], f32, tag="xT32c")
                eng = (nc.sync, nc.gpsimd, nc.scalar)[j % 3]
                eng.dma_start(
                    xg[:], xT32.ap().rearrange("(ko p) t -> p ko t", p=P)[:, :, j * GC:(j + 1) * GC]
                )
                ps = ps_y.tile([E, GC], f32, tag="py")
                for k in range(D // P):
                    nc.tensor.matmul(ps[:], gwT_sb[:, k, :], xg[:, k, :],
                                     start=(k == 0), stop=(k == D // P - 1))
                nc.scalar.activation(scoresT[:, j * GC:(j + 1) * GC], ps[:], Act.Sigmoid)

            # transpose to token-major scores_all [P, 16, E]
            scores_all = gpool.tile([P, T // P, E], f32)
            for t in range(T // P):
                pst = ps_t.tile([P, E], f32, tag="tr")
                nc.tensor.transpose(pst[:], scoresT[:, t * P:(t + 1) * P], ident[:E, :E])
                nc.vector.tensor_copy(scores_all[:, t, :], pst[:])

            NT = T // P  # 16 token tiles
            s_b = gpool.tile([P, NT, E], f32)
            nc.vector.tensor_tensor(s_b[:], scores_all[:],
                                    bias_sb[:, None, :].to_broadcast([P, NT, E]), Alu.add)
            # group scores (max over each group of 4)
            gs = gpool.tile([P, NT, 4], f32)
            nc.vector.tensor_reduce(gs[:], s_b[:].rearrange("p a (g q) -> p a g q", q=4),
                                    axis=mybir.AxisListType.X, op=Alu.max)
            m1 = gpool.tile([P, NT], f32)
            nc.vector.tensor_reduce(m1[:], gs[:], axis=mybir.AxisListType.X, op=Alu.max)
            eq1 = gpool.tile([P, NT, 4], f32)
            nc.vector.tensor_tensor(eq1[:], gs[:], m1[:, :, None].to_broadcast([P, NT, 4]),
                                    Alu.is_equal)
            gs2 = gpool.tile([P, NT, 4], f32)
            nc.vector.tensor_scalar(eq1[:], eq1[:], BIG, None, op0=Alu.mult)
            nc.vector.tensor_tensor(gs2[:], gs[:], eq1[:], Alu.subtract)
            m2 = gpool.tile([P, NT], f32)
            nc.vector.tensor_reduce(m2[:], gs2[:], axis=mybir.AxisListType.X, op=Alu.max)
            keep = gpool.tile([P, NT, 4], f32)
            nc.vector.tensor_tensor(keep[:], gs[:], m2[:, :, None].to_broadcast([P, NT, 4]),
                                    Alu.is_ge)
            # masked scores
            sm = gpool.tile([P, NT, E], f32)
            nc.vector.memset(sm[:], -BIG)
            keepx = gpool.tile([P, NT, E], i32)
            nc.vector.tensor_copy(keepx[:].rearrange("p a (g q) -> p a g q", q=4),
                                  keep[:, :, :, None].to_broadcast([P, NT, 4, 4]))
            nc.vector.copy_predicated(sm[:], keepx[:], s_b[:])
            # iterative 4th-max threshold
            cur = gpool.tile([P, NT, E], f32)
            nc.vector.tensor_copy(cur[:], sm[:])
            mk = None
            for k in range(4):
                mk = gpool.tile([P, NT], f32, tag=f"mk{k}")
                nc.vector.tensor_reduce(mk[:], cur[:], axis=mybir.AxisListType.X, op=Alu.max)
                if k < 3:
                    eqk = gpool.tile([P, NT, E], f32, tag="eqk")
                    nc.vector.tensor_tensor(eqk[:], cur[:],
                                            mk[:, :, None].to_broadcast([P, NT, E]),
                                            Alu.is_equal)
                    nc.vector.tensor_scalar(eqk[:], eqk[:], BIG, None, op0=Alu.mult)
                    nc.vector.tensor_tensor(cur[:], cur[:], eqk[:], Alu.subtract)
            mask4 = gpool.tile([P, NT, E], f32)
            nc.vector.tensor_tensor(mask4[:], sm[:], mk[:, :, None].to_broadcast([P, NT, E]),
                                    Alu.is_ge)
            comb = gpool.tile([P, NT, E], f32)
            nc.vector.tensor_tensor(comb[:], mask4[:], scores_all[:], Alu.mult)

            # local-expert combine weights + masks
            comb2 = gpool.tile([P, NT, EL], f32)
            m01 = gpool.tile([P, NT, EL], f32)
            for le in range(EL):
                tmp = gpool.tile([P, NT, E], f32, tag="seltmp")
                sel = esel_sb[:, le, None, :].to_broadcast([P, NT, E])
                nc.vector.tensor_tensor(tmp[:], comb[:], sel, Alu.mult)
                nc.vector.tensor_reduce(comb2[:, :, le], tmp[:], axis=mybir.AxisListType.X,
                                        op=Alu.add)
                nc.vector.tensor_tensor(tmp[:], mask4[:], sel, Alu.mult)
                nc.vector.tensor_reduce(m01[:, :, le], tmp[:], axis=mybir.AxisListType.X,
                                        op=Alu.add)

            # comb_dram rows (64-wide, cols 0:EL used), batched 4 tiles/DMA
            for tb in range(NT // 4):
                cd = cdp.tile([P, 4, 64], f32, tag="cd")
                nc.vector.memset(cd[:, :, EL:], 0.0)
                nc.vector.tensor_copy(cd[:, :, 0:EL], comb2[:, tb * 4:(tb + 1) * 4, :])
                nc.sync.dma_start(
                    comb_dram[:].rearrange("(o p) d -> p o d", p=P)[:, tb * 4:(tb + 1) * 4, :],
                    cd[:])

            # transpose local masks to expert-major [EL, T] (rows 0:2 of mr4;
            # rows 2:4 hold the inclusive rank scan, bounced to DRAM in one DMA)
            maskT2 = gpool.tile([EL, T], f32, tag="slab8k")
            for t in range(NT):
                psm = ps_t.tile([EL, P], f32, tag="tr")
                nc.tensor.transpose(psm[:], m01[:, t, :], ident[:])
                nc.vector.tensor_copy(maskT2[:, t * P:(t + 1) * P], psm[:])

            # rank scan along tokens
            zsc = const.tile([EL, 1], f32)
            nc.vector.memset(zsc[:], 0.0)
            rank_inc = gpool.tile([EL, T], f32)
            nc.vector.tensor_tensor_scan(rank_inc[:], maskT2[:],
                                         zsc[:].to_broadcast([EL, T]), 0.0,
                                         op0=Alu.add, op1=Alu.add)
            cnt_i = gpool.tile([EL, 1], i32)
            nc.vector.tensor_copy(cnt_i[:], rank_inc[:, T - 1:T])
            cnt_regs = []
            for e in range(EL):
                r = nc.alloc_register(mybir.EngineType.Pool, f"cnt{e}")
                nc.gpsimd.reg_load(r, cnt_i[e:e + 1, 0:1])
                cnt_regs.append(r)

            # replicate mask/rank to a (tq, le, sub) 128-partition layout:
            # partition p = tq*32 + le*16 + s handles tokens [tq*512,(tq+1)*512)
            nc.sync.dma_start(msk_dram[0:EL, :], maskT2[:])
            nc.sync.dma_start(msk_dram[EL:2 * EL, :], rank_inc[:])
            RP = EL * 16
            TQ = 4
            TC = T // TQ  # 512 tokens per partition-group
            sub16i = const.tile([P, 1], i32)
            nc.gpsimd.iota(sub16i[:], pattern=[[0, 1]], base=0, channel_multiplier=1)
            tqs = const.tile([P, 1], i32)
            nc.vector.tensor_scalar(tqs[:], sub16i[:], 4, None, op0=Alu.logical_shift_right)
            nc.vector.tensor_scalar(tqs[:], tqs[:], 3, None, op0=Alu.bitwise_and)
            nc.vector.tensor_scalar(tqs[:], tqs[:], 9, None, op0=Alu.logical_shift_left)
            nc.vector.tensor_scalar(sub16i[:], sub16i[:], 15, None, op0=Alu.bitwise_and)
            sub16 = const.tile([P, 1], f32)
            nc.vector.tensor_copy(sub16[:], sub16i[:])
            # token-id data: tok = tq*512 + f + 1
            tqb = cdp.tile([P, TC], i32, tag="r_i")
            nc.vector.tensor_copy(tqb[:], tqs[:, 0:1].to_broadcast([P, TC]))
            iof = cdp.tile([P, TC], i32, tag="m_i")
            nc.gpsimd.iota(iof[:], pattern=[[1, TC]], base=1, channel_multiplier=0)
            nc.vector.tensor_tensor(tqb[:], tqb[:], iof[:], Alu.add)
            tok16 = const.tile([P, TC], i16)
            nc.vector.tensor_copy(tok16[:], tqb[:])
            # broadcast loads: partition p = le*64 + tq*16 + s
            mrep = cdp.tile([P, TC], f32, tag="mrep")
            rrep = cdp.tile([P, TC], f32, tag="rrep")
            for le in range(EL):
                mv = msk_dram[le][:].rearrange("(q c) -> q c", q=TQ)
                rv = msk_dram[EL + le][:].rearrange("(q c) -> q c", q=TQ)
                nc.sync.dma_start(mrep[le * 64:(le + 1) * 64, :],
                                  mv[:, None, :].to_broadcast([TQ, 16, TC]))
                nc.sync.dma_start(rrep[le * 64:(le + 1) * 64, :],
                                  rv[:, None, :].to_broadcast([TQ, 16, TC]))
            r_i = cdp.tile([P, TC], i32, tag="r_i")
            nc.vector.tensor_copy(r_i[:], rrep[:])
            m_i = cdp.tile([P, TC], i32, tag="m_i")
            nc.vector.tensor_copy(m_i[:], mrep[:])
            nc.vector.tensor_tensor(r_i[:], r_i[:], m_i[:], Alu.subtract)
            rmod = cdp.tile([P, TC], i32, tag="rmod")
            nc.vector.tensor_scalar(rmod[:], r_i[:], 15, None, op0=Alu.bitwise_and)
            c1 = cdp.tile([P, TC], i32, tag="c1")
            nc.vector.tensor_scalar(c1[:], rmod[:], sub16[:, 0:1], None, op0=Alu.is_equal)
            nc.vector.tensor_tensor(c1[:], c1[:], m_i[:], Alu.bitwise_and)
            rdiv = cdp.tile([P, TC], i32, tag="rdiv")
            nc.vector.tensor_scalar(rdiv[:], r_i[:], 4, None, op0=Alu.logical_shift_right)
            gd = cdp.tile([P, TC], i32, tag="gd")
            nc.vector.tensor_scalar(gd[:], rdiv[:], CW, None, op0=Alu.is_lt)
            nc.vector.tensor_tensor(c1[:], c1[:], gd[:], Alu.bitwise_and)
            nc.vector.tensor_scalar(rdiv[:], rdiv[:], 1, None, op0=Alu.add)
            nc.vector.tensor_tensor(c1[:], c1[:], rdiv[:], Alu.mult)
            nc.vector.tensor_scalar(c1[:], c1[:], 1, None, op0=Alu.subtract)
            idx16 = gpool.tile([P, TC], i16)
            nc.vector.tensor_copy(idx16[:], c1[:])
            gth4 = gpool.tile([P, CW], i16)
            nc.gpsimd.local_scatter(gth4[:], tok16[:], idx16[:],
                                    channels=P, num_elems=CW, num_idxs=TC)
            # merge the 4 token-quarter shards: accumulate into gth_dram
            g4d = dram.tile([EL, TQ, 16, CW], i16)
            nc.sync.dma_start(g4d[:], gth4[:])
            gthm = gpool.tile([32 * EL, CW], i16)
            for le in range(EL):
                gm = cdp.tile([16, CW, TQ], i16, tag=f"gm{le}")
                nc.sync.dma_start(
                    gm[:], g4d[le].rearrange("q s c -> s c q"))
                gsum = cdp.tile([16, CW], i32, tag=f"gsum{le}")
                with nc.allow_low_precision("shard merge: exact small ints"):
                    nc.vector.tensor_reduce(gsum[:], gm[:], axis=mybir.AxisListType.X,
                                            op=Alu.add)
                nc.vector.tensor_copy(gthm[le * 32:le * 32 + 16, :], gsum[:])
                nc.sync.dma_start(gth_dram[le], gthm[le * 32:le * 32 + 16, :])
            gthx = []
            for e in range(EL):
                g = gpool.tile([P, CW], i16, tag=f"gthx{e}")
                nc.sync.dma_start(g[:], gth_dram[e][None, :, :].to_broadcast([8, 16, CW]))
                nc.vector.tensor_scalar(g[:], g[:], 1, None, op0=Alu.subtract)
                gthx.append(g)


        else:
            gthx = []
            for e in range(EL):
                g = gpool.tile([P, CW], i16, tag=f"gthx{e}")
                nc.vector.memset(g[:], 0)
                gthx.append(g)
            combg_stub = gpool.tile([P, 64], f32)
            nc.vector.memset(combg_stub[:], 0.0)
            for t in range((T // P)):
                nc.sync.dma_start(comb_dram[t * P:(t + 1) * P, :], combg_stub[:])
            cnt_regs = []
            for e in range(EL):
                r = nc.alloc_register(mybir.EngineType.Pool, f"cnt{e}")
                nc.gpsimd.reg_mov(r, C)
                cnt_regs.append(r)
        # y_dram zero-init (needed before first scatter_add)
        zero_sb = const.tile([P, D], f16)
        nc.vector.memset(zero_sb[:], 0.0)
        for o in range(4):
            nc.scalar.dma_start(
                y_dram[:].rearrange("(o p) d -> p o d", p=P)[:, o * 4:(o + 1) * 4, :],
                zero_sb[:, None, :].to_broadcast([P, 4, D]),
            )

        # ---------------- shared expert (independent of gate) ----------------
        hsT = gpool.tile([P, II // P, TS], f16, tag="hsT")
        for ic in range(II // P):
            p1 = ps_h.tile([P, TS], f32, tag="p1")
            p3 = ps_h.tile([P, TS], f32, tag="p3")
            for k in range(D // P):
                nc.tensor.matmul(p1[:], ws1_sb[:, k, ic * P:(ic + 1) * P], xTs_sb[:, k, :],
                                 start=(k == 0), stop=(k == D // P - 1))
            for k in range(D // P):
                nc.tensor.matmul(p3[:], ws3_sb[:, k, ic * P:(ic + 1) * P], xTs_sb[:, k, :],
                                 start=(k == 0), stop=(k == D // P - 1))
            s1 = spool.tile([P, TS], f32, tag="sh_s1")
            if USE_SILU:
                nc.scalar.activation(s1[:], p1[:], Act.Silu)
            else:
                nc.scalar.activation(s1[:], p1[:], Act.Sigmoid)
                nc.vector.tensor_tensor(s1[:], s1[:], p1[:], Alu.mult)
            nc.vector.tensor_tensor(hsT[:, ic, :], s1[:], p3[:], Alu.mult)
        zsb = gpool.tile([P, TS // P, D], f32, tag="zsb")
        for t2 in range(TS // P):
            for dc in range(D // 512):
                pz = ps_y.tile([P, 512], f32, tag="py")
                for ic in range(II // P):
                    nc.tensor.matmul(pz[:], hsT[:, ic, t2 * P:(t2 + 1) * P],
                                     ws2_sb[:, ic, dc * 512:(dc + 1) * 512],
                                     start=(ic == 0), stop=(ic == II // P - 1))
                nc.vector.tensor_copy(zsb[:, t2, dc * 512:(dc + 1) * 512], pz[:])

        # ---------------- routed experts ----------------
        skip_experts = ABLATE == 'experts'
        w1_sb = []
        w3_sb = []
        w2_sb = []
        for e in range(EL):
            a = wpool.tile([P, D // P, II], f16, tag=f"w1_{e}")
            nc.scalar.dma_start(a[:], w1T[e].rearrange("(ko p) i -> p ko i", p=P))
            w1_sb.append(a)
            b = wpool.tile([P, D // P, II], f16, tag=f"w3_{e}")
            nc.scalar.dma_start(b[:], w3T[e].rearrange("(ko p) i -> p ko i", p=P))
            w3_sb.append(b)
            c = wpool.tile([P, II // P, D], f16, tag=f"w2_{e}")
            nc.scalar.dma_start(c[:], w2T[e].rearrange("(ko p) d -> p ko d", p=P))
            w2_sb.append(c)
        for e in range(EL) if not skip_experts else []:
            xgT = xpool.tile([P, D // P, C], f16, tag="xgT")
            nc.gpsimd.dma_gather(xgT[:], x16[:], gthx[e][:], num_idxs=C,
                                 num_idxs_reg=cnt_regs[e], elem_size=D,
                                 transpose=True, queue_num=0)
            combg = xpool.tile([P, C // P, 64], f32, tag="combg")
            nc.gpsimd.dma_gather(combg[:], comb_dram[:], gthx[e][:], num_idxs=C,
                                 num_idxs_reg=cnt_regs[e], elem_size=64,
                                 transpose=False, queue_num=0)
            hT = hpool.tile([P, II // P, C], f16, tag="hT")
            for cc0 in range(0, C, 512):
                cw = min(512, C - cc0)
                for ic in range(II // P):
                    p1 = ps_h.tile([P, 512], f32, tag="p1")
                    p3 = ps_h.tile([P, 512], f32, tag="p3")
                    for k in range(D // P):
                        nc.tensor.matmul(p1[:, :cw], w1_sb[e][:, k, ic * P:(ic + 1) * P],
                                         xgT[:, k, cc0:cc0 + cw],
                                         start=(k == 0), stop=(k == D // P - 1))
                    for k in range(D // P):
                        nc.tensor.matmul(p3[:, :cw], w3_sb[e][:, k, ic * P:(ic + 1) * P],
                                         xgT[:, k, cc0:cc0 + cw],
                                         start=(k == 0), stop=(k == D // P - 1))
                    s1 = hpool.tile([P, 512], f32, tag="e_s1")
                    if USE_SILU:
                        nc.scalar.activation(s1[:, :cw], p1[:, :cw], Act.Silu)
                    else:
                        nc.scalar.activation(s1[:, :cw], p1[:, :cw], Act.Sigmoid)
                        nc.vector.tensor_tensor(s1[:, :cw], s1[:, :cw], p1[:, :cw], Alu.mult)
                    nc.vector.tensor_tensor(hT[:, ic, cc0:cc0 + cw], s1[:, :cw], p3[:, :cw],
                                            Alu.mult)
            yg = ypool.tile([P, C // P, D], f16, tag="yg")
            for c5 in range(C // P):
                for dc in range(D // 512):
                    py = ps_y.tile([P, 512], f32, tag="py")
                    for ic in range(II // P):
                        nc.tensor.matmul(py[:], hT[:, ic, c5 * P:(c5 + 1) * P],
                                         w2_sb[e][:, ic, dc * 512:(dc + 1) * 512],
                                         start=(ic == 0), stop=(ic == II // P - 1))
                    nc.vector.tensor_scalar(yg[:, c5, dc * 512:(dc + 1) * 512], py[:],
                                            combg[:, c5, e:e + 1], None, op0=Alu.mult)
            nc.gpsimd.dma_scatter_add(y_dram[:], yg[:], gthx[e][:], num_idxs=C,
                                      num_idxs_reg=cnt_regs[e], elem_size=D,
                                      queue_num=0)

        # ---------------- cross-core reduce + finish ----------------
        if n_cores > 1:
            nc.gpsimd.collective_compute(
                "ReduceScatter", Alu.add,
                replica_groups=[list(range(n_cores))],
                ins=[y_dram[:].opt()],
                outs=[rs_out[:].opt()],
            )
        else:
            # single-core build (simulator validation): take core 0's slice
            nc.sync.dma_start(rs_out[:], y_dram[0:TS, :])
        for t2 in range(TS // P):
            rs_sb = spool.tile([P, D], f16, tag="rs_sb")
            nc.sync.dma_start(rs_sb[:], rs_out[t2 * P:(t2 + 1) * P, :])
            fin = spool.tile([P, D], f32, tag="fin")
            nc.vector.tensor_tensor(fin[:], zsb[:, t2, :], rs_sb[:], Alu.add)
            nc.sync.dma_start(out[t2 * P:(t2 + 1) * P, :], fin[:])


_NC_CACHE = {}


def _get_nc(n_cores=NCORES):
    if n_cores not in _NC_CACHE:
        _NC_CACHE[n_cores] = build_kernel(n_cores)
    return _NC_CACHE[n_cores]


def make_in_maps(inputs, n_cores=NCORES):
    x = np.asarray(inputs["x"], np.float32).reshape(T, D)
    gate_w = np.asarray(inputs["gate_w"], np.float32)
    gate_bias = np.asarray(inputs["gate_bias"], np.float32)
    w1 = np.asarray(inputs["w1"], np.float32)
    w2 = np.asarray(inputs["w2"], np.float32)
    w3 = np.asarray(inputs["w3"], np.float32)
    ws1 = np.asarray(inputs["ws1"], np.float32)
    ws2 = np.asarray(inputs["ws2"], np.float32)
    ws3 = np.asarray(inputs["ws3"], np.float32)

    common = {
        "x16": x.astype(np.float16),
        "xT32": np.ascontiguousarray(x.T),
        "gwT": np.ascontiguousarray(gate_w.T),
        "gb": gate_bias.reshape(1, E),
        "ws1T": np.ascontiguousarray(ws1.T.astype(np.float16)),
        "ws3T": np.ascontiguousarray(ws3.T.astype(np.float16)),
        "ws2T": np.ascontiguousarray(ws2.T.astype(np.float16)),
    }
    in_maps = []
    for c in range(n_cores):
        e0 = (c * EL) % E
        sel = np.zeros((EL, E), np.float32)
        for le in range(EL):
            sel[le, e0 + le] = 1.0
        m = dict(common)
        m["esel"] = sel
        m["w1T"] = np.ascontiguousarray(
            w1[e0:e0 + EL].transpose(0, 2, 1).astype(np.float16))
        m["w3T"] = np.ascontiguousarray(
            w3[e0:e0 + EL].transpose(0, 2, 1).astype(np.float16))
        m["w2T"] = np.ascontiguousarray(
            w2[e0:e0 + EL].transpose(0, 2, 1).astype(np.float16))
        m["xTs"] = np.ascontiguousarray(x[c * TS:(c + 1) * TS].T.astype(np.float16))
        in_maps.append(m)
    return in_maps


def run_traced(inputs, trace=False, **kw):
    from concourse.bass_utils import run_bass_kernel_spmd

    nc = _get_nc(NCORES)
    in_maps = make_in_maps(inputs, NCORES)
    res = run_bass_kernel_spmd(nc, in_maps, core_ids=list(range(NCORES)),
                               trace=trace, **kw)
    slices = [res.results[c]["out"] for c in range(NCORES)]
    y = np.concatenate(slices, axis=0).reshape(*np.asarray(inputs["x"]).shape)
    return y.astype(np.float32), res


def kernel(**inputs) -> np.ndarray:
    return run_traced(inputs)[0]



# revision 14
# speedup vs baseline: 1.1516x; 1.1516x over previous
"""Trainium2 Bass kernel for nn_MoE_89498528514729 (moe_routing).

Expert-parallel sparse MoE across 8 NeuronCores:
  - every core gets the full x; routed experts are sharded 2-per-core
  - gate scores via fp32r matmul (full fp32 precision, 1 cycle/row)
  - group-limited top-4 routing computed token-major on DVE
  - per-expert token ranks via PE prefix-sum matmuls (triangular masks)
  - dispatch tables built with local_scatter; shard-merge via PE matmul
  - per-expert token gather via dma_gather (transposed, fp16)
  - SwiGLU expert FFN in fp16 (fp32 PSUM accumulation), capacity 576
  - weighted outputs scatter-added into a token-ordered partial-sum buffer
  - ReduceScatter combines partials across cores; each core finishes its
    256-token slice by adding the (token-sliced) shared expert output
Host side only shards/transposes/casts inputs and concatenates outputs.
"""

import numpy as np

import concourse.bass as bass
import concourse.mybir as mybir
import concourse.tile as tile
from concourse import bacc

P = 128
T = 2048
D = 1024
II = 512
E = 16
EL = 2            # experts per core
NCORES = 8
TS = T // NCORES  # tokens per core output slice
C = 576           # per-expert compute capacity (actual max count 553)
CG = 640          # gather/scatter capacity (num_idxs must be 128-multiple)
CW = CG // 16     # wrapped index width
NT = T // P       # 16 token tiles
GC = 256          # gate chunk (tokens; fp32r needs >=256 for 1 cyc/row)
NGC = T // GC     # 4 chunks
TQ = 4            # token quarters for local_scatter layout
TC = T // TQ      # 512 tokens per quarter
BIG = 1.0e30

f32 = mybir.dt.float32
f32r = mybir.dt.float32r
f16 = mybir.dt.float16
i16 = mybir.dt.int16
i32 = mybir.dt.int32
Alu = mybir.AluOpType
Act = mybir.ActivationFunctionType


def build_kernel(n_cores: int = NCORES):
    nc = bacc.Bacc("TRN2", target_bir_lowering=False, debug=False, num_devices=n_cores)

    t_ = {}
    def inp(name, shape, dt):
        t_[name] = nc.dram_tensor(name, shape, dt, kind="ExternalInput")

    inp("x16", [T, D], f16)
    inp("xT32", [D, T], f32r)
    inp("gwT", [D, E], f32r)
    inp("gb", [1, E], f32)
    inp("esel", [EL, E], f32)
    inp("w1T", [EL, D, II], f16)
    inp("w3T", [EL, D, II], f16)
    inp("w2T", [EL, II, D], f16)
    inp("ws1T", [D, II], f16)
    inp("ws3T", [D, II], f16)
    inp("ws2T", [II, D], f16)
    inp("xTs", [D, TS], f16)
    inp("identf32", [E, E], f32)
    inp("identf16", [P, P], f16)
    inp("ltri", [P, P], f16)        # ltri[q, p] = q <= p
    inp("lse", [32, 32], f16)       # [(t' e'), (t e)] = (e'==e) & (t'<t)
    inp("selcnt", [32, EL], f16)    # [(t' e'), le] = (e'==le)
    inp("selmrg", [P, 32], f16)     # [(tq le s), (le' s')] = (le==le')&(s==s')
    inp("selrep", [EL, 32, P], f16)  # [le][(le' s), p] = (le'==le)&(s==p%16)
    inp("tok16", [P, TC], i16)      # tq(p)*TC + f + 1
    inp("sub16", [P, 1], f32)       # p % 16
    t_["out"] = nc.dram_tensor("out", [TS, D], f32, kind="ExternalOutput")

    with tile.TileContext(nc) as tc:
        _body(nc, tc, n_cores, t_)
    nc.compile()
    return nc


def _body(nc, tc, n_cores, t_):
    x16, xT32, gwT, gb, esel = t_["x16"], t_["xT32"], t_["gwT"], t_["gb"], t_["esel"]
    w1T, w3T, w2T = t_["w1T"], t_["w3T"], t_["w2T"]
    ws1T, ws3T, ws2T, xTs, out = t_["ws1T"], t_["ws3T"], t_["ws2T"], t_["xTs"], t_["out"]

    import contextlib
    ctx = contextlib.ExitStack()
    with ctx:
        const = ctx.enter_context(tc.tile_pool(name="const", bufs=1))
        wpool = ctx.enter_context(tc.tile_pool(name="wpool", bufs=1))
        gpool = ctx.enter_context(tc.tile_pool(name="gpool", bufs=1))
        spool = ctx.enter_context(tc.tile_pool(name="spool", bufs=2))
        xcp = ctx.enter_context(tc.tile_pool(name="xcp", bufs=2))
        xpool = ctx.enter_context(tc.tile_pool(name="xpool", bufs=2))
        hpool = ctx.enter_context(tc.tile_pool(name="hpool", bufs=1))
        ypool = ctx.enter_context(tc.tile_pool(name="ypool", bufs=2))
        ps_t = ctx.enter_context(tc.tile_pool(name="ps_t", bufs=2, space="PSUM"))
        ps_h = ctx.enter_context(tc.tile_pool(name="ps_h", bufs=2, space="PSUM"))
        ps_y = ctx.enter_context(tc.tile_pool(name="ps_y", bufs=2, space="PSUM"))
        dram = ctx.enter_context(tc.tile_pool(name="dram", bufs=1, space="DRAM"))

        # ---------------- DRAM internals ----------------
        comb_dram = dram.tile([T, 64], f32)
        g2_dram = dram.tile([2, 32, P], f32)   # planes: mask, rank; (e, t) rows
        y_dram = dram.tile([T, D], f16)
        rs_out = dram.tile([TS, D], f16)

        # ---------------- constant & weight loads (SP queue, priority order) --
        ident16 = const.tile([P, P], f16)
        nc.sync.dma_start(ident16[:], t_["identf16"][:, :])
        identg = const.tile([E, E], f32)
        nc.sync.dma_start(identg[:], t_["identf32"][:, :])
        ltri_sb = const.tile([P, P], f16)
        nc.sync.dma_start(ltri_sb[:], t_["ltri"][:, :])
        lse_sb = const.tile([32, 32], f16)
        nc.sync.dma_start(lse_sb[:], t_["lse"][:, :])
        selcnt_sb = const.tile([32, EL], f16)
        nc.sync.dma_start(selcnt_sb[:], t_["selcnt"][:, :])
        selmrg_sb = const.tile([P, 32], f16)
        nc.sync.dma_start(selmrg_sb[:], t_["selmrg"][:, :])
        selrep_sb = const.tile([32, EL, P], f16)
        nc.sync.dma_start(selrep_sb[:], t_["selrep"].ap().rearrange("e k p -> k e p"))
        tok16_sb = const.tile([P, TC], i16)
        nc.sync.dma_start(tok16_sb[:], t_["tok16"][:, :])
        sub16_sb = const.tile([P, 1], f32)
        nc.sync.dma_start(sub16_sb[:], t_["sub16"][:, :])
        bias_sb = const.tile([P, E], f32)
        nc.sync.dma_start(bias_sb[:], gb[0:1, :].to_broadcast([P, E]))
        esel_sb = const.tile([P, EL, E], f32)
        nc.sync.dma_start(esel_sb[:], esel[None, :, :].to_broadcast([P, EL, E]))
        gwT_sb = const.tile([P, D // P, E], f32r)
        nc.sync.dma_start(gwT_sb[:], gwT.ap().rearrange("(ko p) e -> p ko e", p=P))

        # zero tile for y_dram init (DVE, early)
        zero_sb = const.tile([P, D], f16)
        nc.vector.memset(zero_sb[:], 0.0)

        # ---------------- gate: scores chunks + transpose to token-major -----
        scores_all = gpool.tile([P, NT, E], f32)
        xg_chunks = []
        for j in range(NGC):
            xg = xcp.tile([P, D // P, GC], f32r, tag="xgc")
            nc.sync.dma_start(
                xg[:], xT32.ap().rearrange("(ko p) t -> p ko t", p=P)[:, :, j * GC:(j + 1) * GC]
            )
            ps = ps_y.tile([P, GC], f32, tag="py")
            for k in range(D // P):
                nc.tensor.matmul(ps[:E, :],
                                 gwT_sb[:, k, :],
                                 xg[:, k, :],
                                 start=(k == 0), stop=(k == D // P - 1))
            sc = spool.tile([E, GC], f32, tag="scc")
            nc.scalar.activation(sc[:], ps[:E, :], Act.Sigmoid)
            for tt in range(GC // P):
                pst = ps_t.tile([P, E], f32, tag="tr")
                nc.tensor.transpose(pst[:], sc[:, tt * P:(tt + 1) * P], identg[:])
                nc.vector.tensor_copy(scores_all[:, j * (GC // P) + tt, :], pst[:])

        # remaining priority loads on SP: shared weights, then expert-0 weights
        ws1_sb = wpool.tile([P, D // P, II], f16, tag="ws1")
        nc.sync.dma_start(ws1_sb[:], ws1T.ap().rearrange("(ko p) i -> p ko i", p=P))
        ws3_sb = wpool.tile([P, D // P, II], f16, tag="ws3")
        nc.sync.dma_start(ws3_sb[:], ws3T.ap().rearrange("(ko p) i -> p ko i", p=P))
        ws2_sb = wpool.tile([P, II // P, D], f16, tag="ws2")
        nc.sync.dma_start(ws2_sb[:], ws2T.ap().rearrange("(ko p) d -> p ko d", p=P))
        xTs_sb = wpool.tile([P, D // P, TS], f16, tag="xTs")
        nc.sync.dma_start(xTs_sb[:], xTs.ap().rearrange("(ko p) t -> p ko t", p=P))
        w1_sb = [wpool.tile([P, D // P, II], f16, tag=f"w1_{e}", name=f"w1_{e}")
                 for e in range(EL)]
        w3_sb = [wpool.tile([P, D // P, II], f16, tag=f"w3_{e}", name=f"w3_{e}")
                 for e in range(EL)]
        w2_sb = [wpool.tile([P, II // P, D], f16, tag=f"w2_{e}", name=f"w2_{e}")
                 for e in range(EL)]
        nc.sync.dma_start(w1_sb[0][:], w1T[0].rearrange("(ko p) i -> p ko i", p=P))
        nc.sync.dma_start(w3_sb[0][:], w3T[0].rearrange("(ko p) i -> p ko i", p=P))

        # ---------------- routing: group-limited top-4, token-major ----------
        # processed in halves so the first half overlaps later gate-chunk DMAs
        s_b = gpool.tile([P, NT, E], f32)
        sm = gpool.tile([P, NT, E], f32)
        mask4 = gpool.tile([P, NT, E], f32)
        comb = gpool.tile([P, NT, E], f32)
        for h0 in range(2):
            a, b = h0 * (NT // 2), (h0 + 1) * (NT // 2)
            w = b - a
            sb_ = s_b[:, a:b, :]
            nc.vector.tensor_tensor(sb_, scores_all[:, a:b, :],
                                    bias_sb[:, None, :].to_broadcast([P, w, E]), Alu.add)
            gs = gpool.tile([P, NT // 2, 4], f32, tag="gs")
            nc.vector.tensor_reduce(gs[:], sb_.rearrange("p a (g q) -> p a g q", q=4),
                                    axis=mybir.AxisListType.X, op=Alu.max)
            m1 = gpool.tile([P, NT // 2], f32, tag="m1")
            nc.vector.tensor_reduce(m1[:], gs[:], axis=mybir.AxisListType.X, op=Alu.max)
            eq1 = gpool.tile([P, NT // 2, 4], f32, tag="eq1")
            nc.vector.tensor_tensor(eq1[:], gs[:], m1[:, :, None].to_broadcast([P, w, 4]),
                                    Alu.is_equal)
            nc.vector.tensor_scalar(eq1[:], eq1[:], BIG, None, op0=Alu.mult)
            gs2 = gpool.tile([P, NT // 2, 4], f32, tag="gs2")
            nc.vector.tensor_tensor(gs2[:], gs[:], eq1[:], Alu.subtract)
            m2 = gpool.tile([P, NT // 2], f32, tag="m2")
            nc.vector.tensor_reduce(m2[:], gs2[:], axis=mybir.AxisListType.X, op=Alu.max)
            keep = gpool.tile([P, NT // 2, 4], f32, tag="keep")
            nc.vector.tensor_tensor(keep[:], gs[:], m2[:, :, None].to_broadcast([P, w, 4]),
                                    Alu.is_ge)
            sm_ = sm[:, a:b, :]
            nc.vector.memset(sm_, -BIG)
            keepx = gpool.tile([P, NT // 2, E], i32, tag="keepx")
            nc.vector.tensor_copy(keepx[:].rearrange("p a (g q) -> p a g q", q=4),
                                  keep[:, :, :, None].to_broadcast([P, w, 4, 4]))
            nc.vector.copy_predicated(sm_, keepx[:], sb_)
            cur = gpool.tile([P, NT // 2, E], f32, tag="cur")
            nc.vector.tensor_copy(cur[:], sm_)
            mk = None
            for k in range(4):
                mk = gpool.tile([P, NT // 2], f32, tag=f"mk{k}")
                nc.vector.tensor_reduce(mk[:], cur[:], axis=mybir.AxisListType.X, op=Alu.max)
                if k < 3:
                    eqk = gpool.tile([P, NT // 2, E], f32, tag="eqk")
                    nc.vector.tensor_tensor(eqk[:], cur[:],
                                            mk[:, :, None].to_broadcast([P, w, E]),
                                            Alu.is_equal)
                    nc.vector.tensor_scalar(eqk[:], eqk[:], BIG, None, op0=Alu.mult)
                    nc.vector.tensor_tensor(cur[:], cur[:], eqk[:], Alu.subtract)
            nc.vector.tensor_tensor(mask4[:, a:b, :], sm_,
                                    mk[:, :, None].to_broadcast([P, w, E]), Alu.is_ge)
            nc.vector.tensor_tensor(comb[:, a:b, :], mask4[:, a:b, :],
                                    scores_all[:, a:b, :], Alu.mult)

        # local-expert combine weights + masks
        comb2 = gpool.tile([P, NT, EL], f32)
        m01 = gpool.tile([P, NT, EL], f32)
        for le in range(EL):
            tmp = gpool.tile([P, NT, E], f32, tag="seltmp")
            sel = esel_sb[:, le, None, :].to_broadcast([P, NT, E])
            nc.vector.tensor_tensor(tmp[:], comb[:], sel, Alu.mult)
            nc.vector.tensor_reduce(comb2[:, :, le], tmp[:], axis=mybir.AxisListType.X,
                                    op=Alu.add)
            nc.vector.tensor_tensor(tmp[:], mask4[:], sel, Alu.mult)
            nc.vector.tensor_reduce(m01[:, :, le], tmp[:], axis=mybir.AxisListType.X,
                                    op=Alu.add)

        # comb_dram rows (64-wide, cols 0:EL used), one DMA (gpsimd queue)
        cd = gpool.tile([P, NT, 64], f32)
        nc.vector.memset(cd[:, :, EL:], 0.0)
        nc.vector.tensor_copy(cd[:, :, 0:EL], comb2[:])
        nc.gpsimd.dma_start(
            comb_dram[:].rearrange("(o p) d -> p o d", p=P), cd[:])

        # ---------------- ranks via PE prefix-sum matmuls --------------------
        m01f = gpool.tile([P, NT * EL], f16)
        nc.vector.tensor_copy(m01f[:].rearrange("p (e t) -> p t e", e=EL), m01[:])
        ps_incl = ps_t.tile([P, NT * EL], f32, tag="tr")
        nc.tensor.matmul(ps_incl[:], ltri_sb[:], m01f[:], start=True, stop=True)
        inclf = gpool.tile([P, NT * EL], f16)
        nc.vector.tensor_copy(inclf[:], ps_incl[:])
        # transpose incl and m01 to (tl, le)-major
        ps_iT = ps_t.tile([32, P], f16, tag="tr")
        nc.tensor.transpose(ps_iT[:], inclf[:], ident16[:])
        ps_mT = ps_t.tile([32, P], f16, tag="tr")
        nc.tensor.transpose(ps_mT[:], m01f[:], ident16[:])
        mgm = gpool.tile([32, P], f32)
        nc.vector.tensor_copy(mgm[:], ps_mT[:])
        mgr = gpool.tile([32, P], f32)
        nc.vector.tensor_copy(mgr[:], ps_iT[:])
        lastc = gpool.tile([32, 1], f16)
        nc.vector.tensor_copy(lastc[:], mgr[:, P - 1:P])
        ps_off = ps_t.tile([32, 1], f32, tag="tr")
        nc.tensor.matmul(ps_off[:], lse_sb[:], lastc[:], start=True, stop=True)
        off_sb = gpool.tile([32, 1], f32)
        nc.vector.tensor_copy(off_sb[:], ps_off[:])
        nc.vector.tensor_scalar(mgr[:], mgr[:], off_sb[:, 0:1], None, op0=Alu.add)
        ps_cnt = ps_t.tile([EL, 1], f32, tag="tr")
        nc.tensor.matmul(ps_cnt[:], selcnt_sb[:], lastc[:], start=True, stop=True)
        cnt_i = gpool.tile([EL, 1], i32)
        nc.vector.tensor_copy(cnt_i[:], ps_cnt[:])
        cnt_regs = []
        for e in range(EL):
            r = nc.alloc_register(mybir.EngineType.Pool, f"cnt{e}")
            nc.gpsimd.reg_load(r, cnt_i[e:e + 1, 0:1])
            cnt_regs.append(r)

        # bounce (mask, rank) to DRAM; reload replicated to (le, tq, s) layout
        nc.gpsimd.dma_start(g2_dram[0], mgm[:])
        nc.gpsimd.dma_start(g2_dram[1], mgr[:])
        mrep = gpool.tile([P, TC], f32)
        rrep = gpool.tile([P, TC], f32)
        nc.gpsimd.dma_start(
            mrep[:],
            g2_dram[0].rearrange("(g fl) p -> g (fl p)", fl=TQ)
            [:, None, :].to_broadcast([EL * TQ, 16, TC]))
        nc.gpsimd.dma_start(
            rrep[:],
            g2_dram[1].rearrange("(g fl) p -> g (fl p)", fl=TQ)
            [:, None, :].to_broadcast([EL * TQ, 16, TC]))

        # slot indices: partition p=(tq,le,s); token f of quarter tq
        r_i = gpool.tile([P, TC], i32, tag="r_i")
        nc.vector.tensor_copy(r_i[:], rrep[:])
        m_i = gpool.tile([P, TC], i32, tag="m_i")
        nc.vector.tensor_copy(m_i[:], mrep[:])
        nc.vector.tensor_tensor(r_i[:], r_i[:], m_i[:], Alu.subtract)
        rmod = gpool.tile([P, TC], i32, tag="rmod")
        nc.vector.tensor_scalar(rmod[:], r_i[:], 15, None, op0=Alu.bitwise_and)
        c1 = gpool.tile([P, TC], i32, tag="c1")
        nc.vector.tensor_scalar(c1[:], rmod[:], sub16_sb[:, 0:1], None, op0=Alu.is_equal)
        nc.vector.tensor_tensor(c1[:], c1[:], m_i[:], Alu.bitwise_and)
        rdiv = gpool.tile([P, TC], i32, tag="rdiv")
        nc.vector.tensor_scalar(rdiv[:], r_i[:], 4, None, op0=Alu.logical_shift_right)
        gd = gpool.tile([P, TC], i32, tag="gd")
        nc.vector.tensor_scalar(gd[:], rdiv[:], CW, None, op0=Alu.is_lt)
        nc.vector.tensor_tensor(c1[:], c1[:], gd[:], Alu.bitwise_and)
        nc.vector.tensor_scalar(rdiv[:], rdiv[:], 1, None, op0=Alu.add)
        nc.vector.tensor_tensor(c1[:], c1[:], rdiv[:], Alu.mult)
        nc.vector.tensor_scalar(c1[:], c1[:], 1, None, op0=Alu.subtract)
        idx16 = gpool.tile([P, TC], i16)
        nc.vector.tensor_copy(idx16[:], c1[:])
        gth4 = gpool.tile([P, CW], i16)
        nc.gpsimd.local_scatter(gth4[:], tok16_sb[:], idx16[:],
                                channels=P, num_elems=CW, num_idxs=TC)
        # merge the 4 token-quarter shards via PE, then replicate to 128 parts
        gthf = gpool.tile([P, CW], f16)
        nc.vector.tensor_copy(gthf[:], gth4[:])
        ps_mrg = ps_t.tile([32, CW], f32, tag="tr")
        nc.tensor.matmul(ps_mrg[:], selmrg_sb[:], gthf[:], start=True, stop=True)
        mrg_sb = gpool.tile([32, CW], f16)
        nc.vector.tensor_copy(mrg_sb[:], ps_mrg[:])
        gthx = []
        for e in range(EL):
            ps_rep = ps_t.tile([P, CW], f32, tag="tr")
            nc.tensor.matmul(ps_rep[:], selrep_sb[:, e, :], mrg_sb[:],
                             start=True, stop=True)
            g = gpool.tile([P, CW], i16, tag=f"gthx{e}")
            nc.vector.tensor_scalar(g[:], ps_rep[:], 1, None, op0=Alu.subtract)
            gthx.append(g)

        # ---------------- shared expert (h stage; z stage is emitted later) --
        hsT = gpool.tile([P, II // P, TS], f16, tag="hsT")
        for ic in range(II // P):
            p1 = ps_h.tile([P, TS], f32, tag="p1")
            p3 = ps_h.tile([P, TS], f32, tag="p3")
            for k in range(D // P):
                nc.tensor.matmul(p1[:], ws1_sb[:, k, ic * P:(ic + 1) * P], xTs_sb[:, k, :],
                                 start=(k == 0), stop=(k == D // P - 1))
            for k in range(D // P):
                nc.tensor.matmul(p3[:], ws3_sb[:, k, ic * P:(ic + 1) * P], xTs_sb[:, k, :],
                                 start=(k == 0), stop=(k == D // P - 1))
            s1 = spool.tile([P, TS], f32, tag="sh_s1")
            nc.scalar.activation(s1[:], p1[:], Act.Silu)
            nc.vector.tensor_tensor(hsT[:, ic, :], s1[:], p3[:], Alu.mult)

        # ---------------- routed experts -------------------------------------
        for e in range(EL):
            xgT = xpool.tile([P, D // P, CG], f16, tag="xgT")
            nc.gpsimd.dma_gather(xgT[:], x16[:], gthx[e][:], num_idxs=CG,
                                 num_idxs_reg=cnt_regs[e], elem_size=D,
                                 transpose=True, queue_num=0)
            combg = xpool.tile([P, CG // P, 64], f32, tag="combg")
            nc.gpsimd.dma_gather(combg[:], comb_dram[:], gthx[e][:], num_idxs=CG,
                                 num_idxs_reg=cnt_regs[e], elem_size=64,
                                 transpose=False, queue_num=0)
            if e == 0:
                # non-critical loads queue behind the first gathers (gpsimd)
                nc.gpsimd.dma_start(w1_sb[1][:], w1T[1].rearrange("(ko p) i -> p ko i", p=P))
                nc.gpsimd.dma_start(w3_sb[1][:], w3T[1].rearrange("(ko p) i -> p ko i", p=P))
                nc.gpsimd.dma_start(w2_sb[0][:], w2T[0].rearrange("(ko p) d -> p ko d", p=P))
                nc.gpsimd.dma_start(w2_sb[1][:], w2T[1].rearrange("(ko p) d -> p ko d", p=P))
                for o in range(4):
                    nc.gpsimd.dma_start(
                        y_dram[:].rearrange("(o p) d -> p o d", p=P)[:, o * 4:(o + 1) * 4, :],
                        zero_sb[:, None, :].to_broadcast([P, 4, D]),
                    )
            hT = hpool.tile([P, II // P, C], f16, tag="hT")
            for ic in range(II // P):
                for cc0, cw in ((0, 512), (512, C - 512)):
                    p1 = ps_h.tile([P, 512], f32, tag="p1")
                    p3 = ps_h.tile([P, 512], f32, tag="p3")
                    for k in range(D // P):
                        nc.tensor.matmul(p1[:, :cw], w1_sb[e][:, k, ic * P:(ic + 1) * P],
                                         xgT[:, k, cc0:cc0 + cw],
                                         start=(k == 0), stop=(k == D // P - 1))
                    for k in range(D // P):
                        nc.tensor.matmul(p3[:, :cw], w3_sb[e][:, k, ic * P:(ic + 1) * P],
                                         xgT[:, k, cc0:cc0 + cw],
                                         start=(k == 0), stop=(k == D // P - 1))
                    s1 = hpool.tile([P, 512], f32, tag="e_s1")
                    nc.scalar.activation(s1[:, :cw], p1[:, :cw], Act.Silu)
                    nc.vector.tensor_tensor(hT[:, ic, cc0:cc0 + cw], s1[:, :cw],
                                            p3[:, :cw], Alu.mult)
            yg = ypool.tile([P, CG // P, D], f16, tag="yg")
            for c5 in range((C + P - 1) // P):
                pw = min(P, C - c5 * P)
                for dc in range(D // 512):
                    py = ps_y.tile([P, 512], f32, tag="py")
                    for ic in range(II // P):
                        nc.tensor.matmul(py[:pw, :], hT[:, ic, c5 * P:c5 * P + pw],
                                         w2_sb[e][:, ic, dc * 512:(dc + 1) * 512],
                                         start=(ic == 0), stop=(ic == II // P - 1))
                    nc.vector.tensor_scalar(yg[:pw, c5, dc * 512:(dc + 1) * 512],
                                            py[:pw, :], combg[:pw, c5, e:e + 1], None,
                                            op0=Alu.mult)
            nc.gpsimd.dma_scatter_add(y_dram[:], yg[:], gthx[e][:], num_idxs=CG,
                                      num_idxs_reg=cnt_regs[e], elem_size=D,
                                      queue_num=0)

        # ---------------- shared expert z stage (fills PE gap near gathers) --
        zsb = gpool.tile([P, TS // P, D], f16, tag="zsb")
        for t2 in range(TS // P):
            for dc in range(D // 512):
                pz = ps_y.tile([P, 512], f32, tag="py")
                for ic in range(II // P):
                    nc.tensor.matmul(pz[:], hsT[:, ic, t2 * P:(t2 + 1) * P],
                                     ws2_sb[:, ic, dc * 512:(dc + 1) * 512],
                                     start=(ic == 0), stop=(ic == II // P - 1))
                nc.vector.tensor_copy(zsb[:, t2, dc * 512:(dc + 1) * 512], pz[:])

        # ---------------- cross-core reduce + finish ----------------
        if n_cores > 1:
            nc.gpsimd.collective_compute(
                "ReduceScatter", Alu.add,
                replica_groups=[list(range(n_cores))],
                ins=[y_dram[:].opt()],
                outs=[rs_out[:].opt()],
            )
        else:
            # single-core build (simulator validation): take core 0's slice
            nc.sync.dma_start(rs_out[:], y_dram[0:TS, :])
        for t2 in range(TS // P):
            rs_sb = spool.tile([P, D], f16, tag="rs_sb")
            nc.sync.dma_start(rs_sb[:], rs_out[t2 * P:(t2 + 1) * P, :])
            fin = spool.tile([P, D], f32, tag="fin")
            nc.vector.tensor_tensor(fin[:], zsb[:, t2, :], rs_sb[:], Alu.add)
            nc.sync.dma_start(out[t2 * P:(t2 + 1) * P, :], fin[:])


_NC_CACHE = {}


def _get_nc(n_cores=NCORES):
    if n_cores not in _NC_CACHE:
        _NC_CACHE[n_cores] = build_kernel(n_cores)
    return _NC_CACHE[n_cores]


def _host_consts():
    p = np.arange(P)
    q = np.arange(P)
    consts = {}
    consts["identf32"] = np.eye(E, dtype=np.float32)
    consts["identf16"] = np.eye(P, dtype=np.float16)
    consts["ltri"] = (q[:, None] <= p[None, :]).astype(np.float16)
    # rows/cols indexed by (e, t): idx = e*NT + t
    t_of = np.arange(32) % NT
    e_of = np.arange(32) // NT
    consts["lse"] = ((e_of[:, None] == e_of[None, :]) &
                     (t_of[:, None] < t_of[None, :])).astype(np.float16)
    consts["selcnt"] = (e_of[:, None] == np.arange(EL)[None, :]).astype(np.float16)
    # partition p = (le, tq, s): le = p>>6, tq = (p>>4)&3, s = p&15
    tq_p = (p >> 4) & 3
    le_p = p >> 6
    s_p = p & 15
    # cols (le', s'): idx = le'*16 + s'
    le_c = np.arange(32) >> 4
    s_c = np.arange(32) & 15
    consts["selmrg"] = ((le_p[:, None] == le_c[None, :]) &
                        (s_p[:, None] == s_c[None, :])).astype(np.float16)
    selrep = np.zeros((EL, 32, P), np.float16)
    for e in range(EL):
        selrep[e] = ((le_c[:, None] == e) & (s_c[:, None] == (p[None, :] & 15)))
    consts["selrep"] = selrep
    consts["tok16"] = (tq_p[:, None] * TC + np.arange(TC)[None, :] + 1).astype(np.int16)
    consts["sub16"] = s_p[:, None].astype(np.float32)
    return consts


def make_in_maps(inputs, n_cores=NCORES):
    x = np.asarray(inputs["x"], np.float32).reshape(T, D)
    gate_w = np.asarray(inputs["gate_w"], np.float32)
    gate_bias = np.asarray(inputs["gate_bias"], np.float32)
    w1 = np.asarray(inputs["w1"], np.float32)
    w2 = np.asarray(inputs["w2"], np.float32)
    w3 = np.asarray(inputs["w3"], np.float32)
    ws1 = np.asarray(inputs["ws1"], np.float32)
    ws2 = np.asarray(inputs["ws2"], np.float32)
    ws3 = np.asarray(inputs["ws3"], np.float32)

    common = {
        "x16": x.astype(np.float16),
        "xT32": np.ascontiguousarray(x.T),
        "gwT": np.ascontiguousarray(gate_w.T),
        "gb": gate_bias.reshape(1, E),
        "ws1T": np.ascontiguousarray(ws1.T.astype(np.float16)),
        "ws3T": np.ascontiguousarray(ws3.T.astype(np.float16)),
        "ws2T": np.ascontiguousarray(ws2.T.astype(np.float16)),
    }
    common.update(_host_consts())
    in_maps = []
    for c in range(n_cores):
        e0 = (c * EL) % E
        sel = np.zeros((EL, E), np.float32)
        for le in range(EL):
            sel[le, e0 + le] = 1.0
        m = dict(common)
        m["esel"] = sel
        m["w1T"] = np.ascontiguousarray(
            w1[e0:e0 + EL].transpose(0, 2, 1).astype(np.float16))
        m["w3T"] = np.ascontiguousarray(
            w3[e0:e0 + EL].transpose(0, 2, 1).astype(np.float16))
        m["w2T"] = np.ascontiguousarray(
            w2[e0:e0 + EL].transpose(0, 2, 1).astype(np.float16))
        m["xTs"] = np.ascontiguousarray(x[c * TS:(c + 1) * TS].T.astype(np.float16))
        in_maps.append(m)
    return in_maps


def run_traced(inputs, trace=False, **kw):
    from concourse.bass_utils import run_bass_kernel_spmd

    nc = _get_nc(NCORES)
    in_maps = make_in_maps(inputs, NCORES)
    res = run_bass_kernel_spmd(nc, in_maps, core_ids=list(range(NCORES)),
                               trace=trace, **kw)
    slices = [res.results[c]["out"] for c in range(NCORES)]
    y = np.concatenate(slices, axis=0).reshape(*np.asarray(inputs["x"]).shape)
    return y.astype(np.float32), res


def kernel(**inputs) -> np.ndarray:
    return run_traced(inputs)[0]


# revision 16
# speedup vs baseline: 1.2896x; 1.1198x over previous
"""Trainium2 Bass kernel for nn_MoE_89498528514729 (moe_routing).

Expert-parallel sparse MoE across 8 NeuronCores:
  - every core gets the full x; routed experts are sharded 2-per-core
  - gate scores via fp32r matmul (full fp32 precision, 1 cycle/row)
  - group-limited top-4 routing computed token-major on DVE
  - per-expert token ranks via PE prefix-sum matmuls (triangular masks)
  - dispatch tables built with local_scatter; shard-merge via PE matmul
  - per-expert token gather via dma_gather (transposed, fp16)
  - SwiGLU expert FFN in fp16 (fp32 PSUM accumulation), capacity 576
  - weighted outputs scatter-added into a token-ordered partial-sum buffer
  - ReduceScatter combines partials across cores; each core finishes its
    256-token slice by adding the (token-sliced) shared expert output
Host side only shards/transposes/casts inputs and concatenates outputs.
"""

import numpy as np

import concourse.bass as bass
import concourse.mybir as mybir
import concourse.tile as tile
from concourse import bacc
from concourse.tile_rust import add_dep_helper

P = 128
T = 2048
D = 1024
II = 512
E = 16
EL = 2            # experts per core
NCORES = 8
TS = T // NCORES  # tokens per core output slice
C = 576           # per-expert compute capacity (actual max count 553)
CG = 640          # gather/scatter capacity (num_idxs must be 128-multiple)
CW = CG // 16     # wrapped index width
NT = T // P       # 16 token tiles
GC = 256          # gate chunk (tokens; fp32r needs >=256 for 1 cyc/row)
NGC = T // GC     # 4 chunks
TQ = 4            # token quarters for local_scatter layout
TC = T // TQ      # 512 tokens per quarter
BIG = 1.0e30

f32 = mybir.dt.float32
f32r = mybir.dt.float32r
f16 = mybir.dt.float16
i16 = mybir.dt.int16
i32 = mybir.dt.int32
Alu = mybir.AluOpType
Act = mybir.ActivationFunctionType


def build_kernel(n_cores: int = NCORES):
    nc = bacc.Bacc("TRN2", target_bir_lowering=False, debug=False, num_devices=n_cores)

    t_ = {}
    def inp(name, shape, dt):
        t_[name] = nc.dram_tensor(name, shape, dt, kind="ExternalInput")

    inp("x16", [T, D], f16)
    inp("xT32", [D, T], f32r)
    inp("gwT", [D, E], f32r)
    inp("gb", [1, E], f32)
    inp("esel", [EL, E], f32)
    inp("w1T", [EL, D, II], f16)
    inp("w3T", [EL, D, II], f16)
    inp("w2T", [EL, II, D], f16)
    inp("ws1T", [D, II], f16)
    inp("ws3T", [D, II], f16)
    inp("ws2T", [II, D], f16)
    inp("xTs", [D, TS], f16)
    inp("identf32", [E, E], f32)
    inp("identf16", [P, P], f16)
    inp("ltri", [P, P], f16)        # ltri[q, p] = q <= p
    inp("lse", [32, 32], f16)       # [(t' e'), (t e)] = (e'==e) & (t'<t)
    inp("selcnt", [32, EL], f16)    # [(t' e'), le] = (e'==le)
    inp("selmrg", [P, 32], f16)     # [(tq le s), (le' s')] = (le==le')&(s==s')
    inp("selrep", [EL, 32, P], f16)  # [le][(le' s), p] = (le'==le)&(s==p%16)
    inp("tok16", [P, TC], i16)      # tq(p)*TC + f + 1
    inp("sub16", [P, 1], f32)       # p % 16
    t_["out"] = nc.dram_tensor("out", [TS, D], f32, kind="ExternalOutput")

    with tile.TileContext(nc) as tc:
        _body(nc, tc, n_cores, t_)
    nc.compile()
    return nc


def _body(nc, tc, n_cores, t_):
    x16, xT32, gwT, gb, esel = t_["x16"], t_["xT32"], t_["gwT"], t_["gb"], t_["esel"]
    w1T, w3T, w2T = t_["w1T"], t_["w3T"], t_["w2T"]
    ws1T, ws3T, ws2T, xTs, out = t_["ws1T"], t_["ws3T"], t_["ws2T"], t_["xTs"], t_["out"]

    import contextlib
    ctx = contextlib.ExitStack()
    with ctx:
        const = ctx.enter_context(tc.tile_pool(name="const", bufs=1))
        wpool = ctx.enter_context(tc.tile_pool(name="wpool", bufs=1))
        gpool = ctx.enter_context(tc.tile_pool(name="gpool", bufs=1))
        spool = ctx.enter_context(tc.tile_pool(name="spool", bufs=2))
        xcp = ctx.enter_context(tc.tile_pool(name="xcp", bufs=2))
        xpool = ctx.enter_context(tc.tile_pool(name="xpool", bufs=2))
        hpool = ctx.enter_context(tc.tile_pool(name="hpool", bufs=1))
        ypool = ctx.enter_context(tc.tile_pool(name="ypool", bufs=1))
        ps_t = ctx.enter_context(tc.tile_pool(name="ps_t", bufs=2, space="PSUM"))
        ps_h = ctx.enter_context(tc.tile_pool(name="ps_h", bufs=2, space="PSUM"))
        ps_y = ctx.enter_context(tc.tile_pool(name="ps_y", bufs=2, space="PSUM"))
        dram = ctx.enter_context(tc.tile_pool(name="dram", bufs=1, space="DRAM"))

        # ---------------- DRAM internals ----------------
        comb_dram = dram.tile([T, 64], f32)
        g2_dram = dram.tile([32, 3, P], i32)   # rows (e,t); planes m2, rmod, rdiv+1
        y_dram = dram.tile([T, D], f16)
        rs_out = dram.tile([TS, D], f16)

        # ---------------- constant & weight loads (SP queue, priority order) --
        ident16 = const.tile([P, P], f16)
        nc.sync.dma_start(ident16[:], t_["identf16"][:, :])
        identg = const.tile([E, E], f32)
        nc.sync.dma_start(identg[:], t_["identf32"][:, :])
        ltri_sb = const.tile([P, P], f16)
        nc.sync.dma_start(ltri_sb[:], t_["ltri"][:, :])
        lse_sb = const.tile([32, 32], f16)
        nc.sync.dma_start(lse_sb[:], t_["lse"][:, :])
        selcnt_sb = const.tile([32, EL], f16)
        nc.sync.dma_start(selcnt_sb[:], t_["selcnt"][:, :])
        selmrg_sb = const.tile([P, 32], f16)
        nc.sync.dma_start(selmrg_sb[:], t_["selmrg"][:, :])
        selrep_sb = const.tile([32, EL, P], f16)
        nc.sync.dma_start(selrep_sb[:], t_["selrep"].ap().rearrange("e k p -> k e p"))
        tok16_sb = const.tile([P, TC], i16)
        nc.sync.dma_start(tok16_sb[:], t_["tok16"][:, :])
        sub16_sb = const.tile([P, 1], f32)
        nc.sync.dma_start(sub16_sb[:], t_["sub16"][:, :])
        bias_sb = const.tile([P, E], f32)
        nc.sync.dma_start(bias_sb[:], gb[0:1, :].to_broadcast([P, E]))
        esel_sb = const.tile([P, EL, E], f32)
        nc.sync.dma_start(esel_sb[:], esel[None, :, :].to_broadcast([P, EL, E]))
        gwT_sb = const.tile([P, D // P, E], f32r)
        nc.sync.dma_start(gwT_sb[:], gwT.ap().rearrange("(ko p) e -> p ko e", p=P))

        # zero tile for y_dram init (DVE, early)
        zero_sb = const.tile([P, D], f16)
        nc.vector.memset(zero_sb[:], 0.0)

        # ---------------- gate: scores chunks + transpose to token-major -----
        scores_all = gpool.tile([P, NT, E], f32)
        chunk_dmas = []
        for j in range(NGC):
            xg = xcp.tile([P, D // P, GC], f32r, tag="xgc")
            cdma = nc.sync.dma_start(
                xg[:], xT32.ap().rearrange("(ko p) t -> p ko t", p=P)[:, :, j * GC:(j + 1) * GC]
            )
            chunk_dmas.append(cdma)
            ps = ps_y.tile([P, GC], f32, tag="py")
            for k in range(D // P):
                nc.tensor.matmul(ps[:E, :],
                                 gwT_sb[:, k, :],
                                 xg[:, k, :],
                                 start=(k == 0), stop=(k == D // P - 1))
            sc = spool.tile([E, GC], f32, tag="scc")
            nc.scalar.activation(sc[:], ps[:E, :], Act.Sigmoid)
            for tt in range(GC // P):
                pst = ps_t.tile([P, E], f32, tag="tr")
                nc.tensor.transpose(pst[:], sc[:, tt * P:(tt + 1) * P], identg[:])
                nc.vector.tensor_copy(scores_all[:, j * (GC // P) + tt, :], pst[:])

        # bulk loads, fenced behind the gate-chunk DMAs so the serial DMA
        # device serves the gate (critical path) first
        fence7 = chunk_dmas[NGC - 2].ins
        def fenced_load(dst, src, fence):
            d = nc.sync.dma_start(dst, src)
            add_dep_helper(d.ins, fence, reason="DMA priority fence")
            return d
        ws1_sb = wpool.tile([P, D // P, II], f16, tag="ws1")
        fenced_load(ws1_sb[:], ws1T.ap().rearrange("(ko p) i -> p ko i", p=P), fence7)
        ws3_sb = wpool.tile([P, D // P, II], f16, tag="ws3")
        fenced_load(ws3_sb[:], ws3T.ap().rearrange("(ko p) i -> p ko i", p=P), fence7)
        xTs_sb = wpool.tile([P, D // P, TS], f16, tag="xTs")
        fenced_load(xTs_sb[:], xTs.ap().rearrange("(ko p) t -> p ko t", p=P), fence7)
        ws2_sb = wpool.tile([P, II // P, D], f16, tag="ws2")
        fenced_load(ws2_sb[:], ws2T.ap().rearrange("(ko p) d -> p ko d", p=P), fence7)
        w1_sb = [wpool.tile([P, D // P, II], f16, tag=f"w1_{e}", name=f"w1_{e}")
                 for e in range(EL)]
        w3_sb = [wpool.tile([P, D // P, II], f16, tag=f"w3_{e}", name=f"w3_{e}")
                 for e in range(EL)]
        w2_sb = [wpool.tile([P, II // P, D], f16, tag=f"w2_{e}", name=f"w2_{e}")
                 for e in range(EL)]
        fenced_load(w1_sb[0][:], w1T[0].rearrange("(ko p) i -> p ko i", p=P), fence7)
        fenced_load(w3_sb[0][:], w3T[0].rearrange("(ko p) i -> p ko i", p=P), fence7)

        # ---------------- routing: group-limited top-4, token-major ----------
        # processed in halves so the first half overlaps later gate-chunk DMAs
        s_b = gpool.tile([P, NT, E], f32)
        sm = gpool.tile([P, NT, E], f32)
        mask4 = gpool.tile([P, NT, E], f32)
        comb = gpool.tile([P, NT, E], f32)
        for h0 in range(2):
            a, b = h0 * (NT // 2), (h0 + 1) * (NT // 2)
            w = b - a
            sb_ = s_b[:, a:b, :]
            nc.vector.tensor_tensor(sb_, scores_all[:, a:b, :],
                                    bias_sb[:, None, :].to_broadcast([P, w, E]), Alu.add)
            gs = gpool.tile([P, NT // 2, 4], f32, tag="gs")
            nc.vector.tensor_reduce(gs[:], sb_.rearrange("p a (g q) -> p a g q", q=4),
                                    axis=mybir.AxisListType.X, op=Alu.max)
            m1 = gpool.tile([P, NT // 2], f32, tag="m1")
            nc.vector.tensor_reduce(m1[:], gs[:], axis=mybir.AxisListType.X, op=Alu.max)
            eq1 = gpool.tile([P, NT // 2, 4], f32, tag="eq1")
            nc.vector.tensor_tensor(eq1[:], gs[:], m1[:, :, None].to_broadcast([P, w, 4]),
                                    Alu.is_equal)
            nc.vector.tensor_scalar(eq1[:], eq1[:], BIG, None, op0=Alu.mult)
            gs2 = gpool.tile([P, NT // 2, 4], f32, tag="gs2")
            nc.vector.tensor_tensor(gs2[:], gs[:], eq1[:], Alu.subtract)
            m2 = gpool.tile([P, NT // 2], f32, tag="m2")
            nc.vector.tensor_reduce(m2[:], gs2[:], axis=mybir.AxisListType.X, op=Alu.max)
            keep = gpool.tile([P, NT // 2, 4], f32, tag="keep")
            nc.vector.tensor_tensor(keep[:], gs[:], m2[:, :, None].to_broadcast([P, w, 4]),
                                    Alu.is_ge)
            sm_ = sm[:, a:b, :]
            nc.vector.memset(sm_, -BIG)
            keepx = gpool.tile([P, NT // 2, E], i32, tag="keepx")
            nc.vector.tensor_copy(keepx[:].rearrange("p a (g q) -> p a g q", q=4),
                                  keep[:, :, :, None].to_broadcast([P, w, 4, 4]))
            nc.vector.copy_predicated(sm_, keepx[:], sb_)
            cur = gpool.tile([P, NT // 2, E], f32, tag="cur")
            nc.vector.tensor_copy(cur[:], sm_)
            mk = None
            for k in range(4):
                mk = gpool.tile([P, NT // 2], f32, tag=f"mk{k}")
                nc.vector.tensor_reduce(mk[:], cur[:], axis=mybir.AxisListType.X, op=Alu.max)
                if k < 3:
                    eqk = gpool.tile([P, NT // 2, E], f32, tag="eqk")
                    nc.vector.tensor_tensor(eqk[:], cur[:],
                                            mk[:, :, None].to_broadcast([P, w, E]),
                                            Alu.is_equal)
                    nc.vector.tensor_scalar(eqk[:], eqk[:], BIG, None, op0=Alu.mult)
                    nc.vector.tensor_tensor(cur[:], cur[:], eqk[:], Alu.subtract)
            nc.vector.tensor_tensor(mask4[:, a:b, :], sm_,
                                    mk[:, :, None].to_broadcast([P, w, E]), Alu.is_ge)
            nc.vector.tensor_tensor(comb[:, a:b, :], mask4[:, a:b, :],
                                    scores_all[:, a:b, :], Alu.mult)

        # local-expert combine weights + masks
        comb2 = gpool.tile([P, NT, EL], f32)
        m01 = gpool.tile([P, NT, EL], f32)
        for le in range(EL):
            tmp = gpool.tile([P, NT, E], f32, tag="seltmp")
            sel = esel_sb[:, le, None, :].to_broadcast([P, NT, E])
            nc.vector.tensor_tensor(tmp[:], comb[:], sel, Alu.mult)
            nc.vector.tensor_reduce(comb2[:, :, le], tmp[:], axis=mybir.AxisListType.X,
                                    op=Alu.add)
            nc.vector.tensor_tensor(tmp[:], mask4[:], sel, Alu.mult)
            nc.vector.tensor_reduce(m01[:, :, le], tmp[:], axis=mybir.AxisListType.X,
                                    op=Alu.add)

        # comb_dram rows (64-wide, cols 0:EL used), one DMA (gpsimd queue)
        cd = gpool.tile([P, NT, 64], f32)
        nc.vector.memset(cd[:, :, EL:], 0.0)
        nc.vector.tensor_copy(cd[:, :, 0:EL], comb2[:])
        nc.gpsimd.dma_start(
            comb_dram[:].rearrange("(o p) d -> p o d", p=P), cd[:])

        # ---------------- ranks via PE prefix-sum matmuls --------------------
        m01f = gpool.tile([P, NT * EL], f16)
        nc.vector.tensor_copy(m01f[:].rearrange("p (e t) -> p t e", e=EL), m01[:])
        ps_incl = ps_t.tile([P, NT * EL], f32, tag="tr")
        nc.tensor.matmul(ps_incl[:], ltri_sb[:], m01f[:], start=True, stop=True)
        inclf = gpool.tile([P, NT * EL], f16)
        nc.vector.tensor_copy(inclf[:], ps_incl[:])
        # transpose incl and m01 to (tl, le)-major
        ps_iT = ps_t.tile([32, P], f16, tag="tr")
        nc.tensor.transpose(ps_iT[:], inclf[:], ident16[:])
        ps_mT = ps_t.tile([32, P], f16, tag="tr")
        nc.tensor.transpose(ps_mT[:], m01f[:], ident16[:])
        mgm = gpool.tile([32, P], f32)
        nc.vector.tensor_copy(mgm[:], ps_mT[:])
        mgr = gpool.tile([32, P], f32)
        nc.vector.tensor_copy(mgr[:], ps_iT[:])
        lastc = gpool.tile([32, 1], f16)
        nc.vector.tensor_copy(lastc[:], mgr[:, P - 1:P])
        ps_off = ps_t.tile([32, 1], f32, tag="tr")
        nc.tensor.matmul(ps_off[:], lse_sb[:], lastc[:], start=True, stop=True)
        off_sb = gpool.tile([32, 1], f32)
        nc.vector.tensor_copy(off_sb[:], ps_off[:])
        nc.vector.tensor_scalar(mgr[:], mgr[:], off_sb[:, 0:1], None, op0=Alu.add)
        ps_cnt = ps_t.tile([EL, 1], f32, tag="tr")
        nc.tensor.matmul(ps_cnt[:], selcnt_sb[:], lastc[:], start=True, stop=True)
        cnt_i = gpool.tile([EL, 1], i32)
        nc.vector.tensor_copy(cnt_i[:], ps_cnt[:])
        cnt_regs = []
        for e in range(EL):
            r = nc.alloc_register(mybir.EngineType.Pool, f"cnt{e}")
            nc.gpsimd.reg_load(r, cnt_i[e:e + 1, 0:1])
            cnt_regs.append(r)

        # small-side slot arithmetic on [32, P]: planes m2 = m & (r//16 < CW),
        # rmod = r % 16, rdivp1 = r//16 + 1 (r = exclusive rank)
        mga = gpool.tile([32, 3, P], i32)
        ri_s = gpool.tile([32, P], i32)
        nc.vector.tensor_copy(ri_s[:], mgr[:])
        mi_s = gpool.tile([32, P], i32)
        nc.vector.tensor_copy(mi_s[:], mgm[:])
        nc.vector.tensor_tensor(ri_s[:], ri_s[:], mi_s[:], Alu.subtract)
        nc.vector.tensor_scalar(mga[:, 1, :], ri_s[:], 15, None, op0=Alu.bitwise_and)
        rdiv_s = gpool.tile([32, P], i32)
        nc.vector.tensor_scalar(rdiv_s[:], ri_s[:], 4, None,
                                op0=Alu.logical_shift_right)
        gd_s = gpool.tile([32, P], i32)
        nc.vector.tensor_scalar(gd_s[:], rdiv_s[:], CW, None, op0=Alu.is_lt)
        nc.vector.tensor_tensor(mga[:, 0, :], mi_s[:], gd_s[:], Alu.bitwise_and)
        nc.vector.tensor_scalar(mga[:, 2, :], rdiv_s[:], 1, None, op0=Alu.add)

        # one bounce to DRAM; one broadcast reload to (le, tq, s) partitions
        nc.gpsimd.dma_start(g2_dram[:], mga[:])
        mrep3 = gpool.tile([P, TQ, 3, P], i32)
        nc.gpsimd.dma_start(
            mrep3[:].rearrange("pp fl pl p -> pp (fl pl p)"),
            g2_dram[:].rearrange("(g fl) pl p -> g (fl pl p)", fl=TQ)
            [:, None, :].to_broadcast([EL * TQ, 16, TQ * 3 * P]))

        # slot indices: partition p=(le,tq,s); token f=(fl,p2) of quarter tq
        c1 = gpool.tile([P, TQ, P], i32, tag="c1")
        nc.vector.tensor_scalar(c1[:], mrep3[:, :, 1, :], sub16_sb[:, 0:1], None,
                                op0=Alu.is_equal)
        nc.vector.tensor_tensor(c1[:], c1[:], mrep3[:, :, 0, :], Alu.bitwise_and)
        nc.vector.tensor_tensor(c1[:], c1[:], mrep3[:, :, 2, :], Alu.mult)
        nc.vector.tensor_scalar(c1[:], c1[:], 1, None, op0=Alu.subtract)
        idx16 = gpool.tile([P, TC], i16)
        nc.vector.tensor_copy(idx16[:].rearrange("pp (fl p) -> pp fl p", fl=TQ), c1[:])
        gth4 = gpool.tile([P, CW], i16)
        nc.gpsimd.local_scatter(gth4[:], tok16_sb[:], idx16[:],
                                channels=P, num_elems=CW, num_idxs=TC)
        # merge the 4 token-quarter shards via PE, then replicate to 128 parts
        gthf = gpool.tile([P, CW], f16)
        nc.vector.tensor_copy(gthf[:], gth4[:])
        ps_mrg = ps_t.tile([32, CW], f32, tag="tr")
        nc.tensor.matmul(ps_mrg[:], selmrg_sb[:], gthf[:], start=True, stop=True)
        mrg_sb = gpool.tile([32, CW], f16)
        nc.vector.tensor_copy(mrg_sb[:], ps_mrg[:])
        gthx = []
        for e in range(EL):
            ps_rep = ps_t.tile([P, CW], f32, tag="tr")
            nc.tensor.matmul(ps_rep[:], selrep_sb[:, e, :], mrg_sb[:],
                             start=True, stop=True)
            g = gpool.tile([P, CW], i16, tag=f"gthx{e}")
            nc.vector.tensor_scalar(g[:], ps_rep[:], 1, None, op0=Alu.subtract)
            gthx.append(g)

        # ---------------- shared expert (h stage; z stage is emitted later) --
        hsT = gpool.tile([P, II // P, TS], f16, tag="hsT")
        for ic in range(II // P):
            p1 = ps_h.tile([P, TS], f32, tag="p1")
            p3 = ps_h.tile([P, TS], f32, tag="p3")
            for k in range(D // P):
                nc.tensor.matmul(p1[:], ws1_sb[:, k, ic * P:(ic + 1) * P], xTs_sb[:, k, :],
                                 start=(k == 0), stop=(k == D // P - 1))
            for k in range(D // P):
                nc.tensor.matmul(p3[:], ws3_sb[:, k, ic * P:(ic + 1) * P], xTs_sb[:, k, :],
                                 start=(k == 0), stop=(k == D // P - 1))
            s1 = spool.tile([P, TS], f32, tag="sh_s1")
            nc.scalar.activation(s1[:], p1[:], Act.Silu)
            nc.vector.tensor_tensor(hsT[:, ic, :], s1[:], p3[:], Alu.mult)

        # ---------------- routed experts -------------------------------------
        for e in range(EL):
            xgT = xpool.tile([P, D // P, CG], f16, tag="xgT")
            gxg = nc.gpsimd.dma_gather(xgT[:], x16[:], gthx[e][:], num_idxs=CG,
                                       num_idxs_reg=cnt_regs[e], elem_size=D,
                                       transpose=True, queue_num=0)
            combg = xpool.tile([P, CG // P, 64], f32, tag="combg")
            nc.gpsimd.dma_gather(combg[:], comb_dram[:], gthx[e][:], num_idxs=CG,
                                 num_idxs_reg=cnt_regs[e], elem_size=64,
                                 transpose=False, queue_num=0)
            if e == 0:
                # non-critical loads fenced behind the first token gather
                gfence = gxg.ins
                def fenced_load2(dst, srcap):
                    d = nc.scalar.dma_start(dst, srcap)
                    add_dep_helper(d.ins, gfence, reason="DMA priority fence")
                    return d
                fenced_load2(w1_sb[1][:], w1T[1].rearrange("(ko p) i -> p ko i", p=P))
                fenced_load2(w3_sb[1][:], w3T[1].rearrange("(ko p) i -> p ko i", p=P))
                fenced_load2(w2_sb[0][:], w2T[0].rearrange("(ko p) d -> p ko d", p=P))
                fenced_load2(w2_sb[1][:], w2T[1].rearrange("(ko p) d -> p ko d", p=P))
                for o in range(4):
                    fenced_load2(
                        y_dram[:].rearrange("(o p) d -> p o d", p=P)[:, o * 4:(o + 1) * 4, :],
                        zero_sb[:, None, :].to_broadcast([P, 4, D]),
                    )
            hT = hpool.tile([P, II // P, C], f16, tag="hT")
            for ic in range(II // P):
                for cc0, cw in ((0, 512), (512, C - 512)):
                    p1 = ps_h.tile([P, 512], f32, tag="p1")
                    p3 = ps_h.tile([P, 512], f32, tag="p3")
                    for k in range(D // P):
                        nc.tensor.matmul(p1[:, :cw], w1_sb[e][:, k, ic * P:(ic + 1) * P],
                                         xgT[:, k, cc0:cc0 + cw],
                                         start=(k == 0), stop=(k == D // P - 1))
                    for k in range(D // P):
                        nc.tensor.matmul(p3[:, :cw], w3_sb[e][:, k, ic * P:(ic + 1) * P],
                                         xgT[:, k, cc0:cc0 + cw],
                                         start=(k == 0), stop=(k == D // P - 1))
                    s1 = hpool.tile([P, 512], f32, tag="e_s1")
                    nc.scalar.activation(s1[:, :cw], p1[:, :cw], Act.Silu)
                    nc.vector.tensor_tensor(hT[:, ic, cc0:cc0 + cw], s1[:, :cw],
                                            p3[:, :cw], Alu.mult)
            yg = ypool.tile([P, CG // P, D], f16, tag="yg")
            for c5 in range((C + P - 1) // P):
                pw = min(P, C - c5 * P)
                for dc in range(D // 512):
                    py = ps_y.tile([P, 512], f32, tag="py")
                    for ic in range(II // P):
                        nc.tensor.matmul(py[:pw, :], hT[:, ic, c5 * P:c5 * P + pw],
                                         w2_sb[e][:, ic, dc * 512:(dc + 1) * 512],
                                         start=(ic == 0), stop=(ic == II // P - 1))
                    nc.vector.tensor_scalar(yg[:pw, c5, dc * 512:(dc + 1) * 512],
                                            py[:pw, :], combg[:pw, c5, e:e + 1], None,
                                            op0=Alu.mult)
            nc.gpsimd.dma_scatter_add(y_dram[:], yg[:], gthx[e][:], num_idxs=CG,
                                      num_idxs_reg=cnt_regs[e], elem_size=D,
                                      queue_num=0)

        # ---------------- shared expert z stage (fills PE gap near gathers) --
        zsb = gpool.tile([P, TS // P, D], f16, tag="zsb")
        for t2 in range(TS // P):
            for dc in range(D // 512):
                pz = ps_y.tile([P, 512], f32, tag="py")
                for ic in range(II // P):
                    nc.tensor.matmul(pz[:], hsT[:, ic, t2 * P:(t2 + 1) * P],
                                     ws2_sb[:, ic, dc * 512:(dc + 1) * 512],
                                     start=(ic == 0), stop=(ic == II // P - 1))
                nc.vector.tensor_copy(zsb[:, t2, dc * 512:(dc + 1) * 512], pz[:])

        # ---------------- cross-core reduce + finish ----------------
        if n_cores > 1:
            nc.gpsimd.collective_compute(
                "ReduceScatter", Alu.add,
                replica_groups=[list(range(n_cores))],
                ins=[y_dram[:].opt()],
                outs=[rs_out[:].opt()],
            )
        else:
            # single-core build (simulator validation): take core 0's slice
            nc.sync.dma_start(rs_out[:], y_dram[0:TS, :])
        for t2 in range(TS // P):
            rs_sb = spool.tile([P, D], f16, tag="rs_sb")
            nc.sync.dma_start(rs_sb[:], rs_out[t2 * P:(t2 + 1) * P, :])
            fin = spool.tile([P, D], f32, tag="fin")
            nc.vector.tensor_tensor(fin[:], zsb[:, t2, :], rs_sb[:], Alu.add)
            nc.sync.dma_start(out[t2 * P:(t2 + 1) * P, :], fin[:])


_NC_CACHE = {}


def _get_nc(n_cores=NCORES):
    if n_cores not in _NC_CACHE:
        _NC_CACHE[n_cores] = build_kernel(n_cores)
    return _NC_CACHE[n_cores]


def _host_consts():
    p = np.arange(P)
    q = np.arange(P)
    consts = {}
    consts["identf32"] = np.eye(E, dtype=np.float32)
    consts["identf16"] = np.eye(P, dtype=np.float16)
    consts["ltri"] = (q[:, None] <= p[None, :]).astype(np.float16)
    # rows/cols indexed by (e, t): idx = e*NT + t
    t_of = np.arange(32) % NT
    e_of = np.arange(32) // NT
    consts["lse"] = ((e_of[:, None] == e_of[None, :]) &
                     (t_of[:, None] < t_of[None, :])).astype(np.float16)
    consts["selcnt"] = (e_of[:, None] == np.arange(EL)[None, :]).astype(np.float16)
    # partition p = (le, tq, s): le = p>>6, tq = (p>>4)&3, s = p&15
    tq_p = (p >> 4) & 3
    le_p = p >> 6
    s_p = p & 15
    # cols (le', s'): idx = le'*16 + s'
    le_c = np.arange(32) >> 4
    s_c = np.arange(32) & 15
    consts["selmrg"] = ((le_p[:, None] == le_c[None, :]) &
                        (s_p[:, None] == s_c[None, :])).astype(np.float16)
    selrep = np.zeros((EL, 32, P), np.float16)
    for e in range(EL):
        selrep[e] = ((le_c[:, None] == e) & (s_c[:, None] == (p[None, :] & 15)))
    consts["selrep"] = selrep
    consts["tok16"] = (tq_p[:, None] * TC + np.arange(TC)[None, :] + 1).astype(np.int16)
    consts["sub16"] = s_p[:, None].astype(np.float32)
    return consts


def make_in_maps(inputs, n_cores=NCORES):
    x = np.asarray(inputs["x"], np.float32).reshape(T, D)
    gate_w = np.asarray(inputs["gate_w"], np.float32)
    gate_bias = np.asarray(inputs["gate_bias"], np.float32)
    w1 = np.asarray(inputs["w1"], np.float32)
    w2 = np.asarray(inputs["w2"], np.float32)
    w3 = np.asarray(inputs["w3"], np.float32)
    ws1 = np.asarray(inputs["ws1"], np.float32)
    ws2 = np.asarray(inputs["ws2"], np.float32)
    ws3 = np.asarray(inputs["ws3"], np.float32)

    common = {
        "x16": x.astype(np.float16),
        "xT32": np.ascontiguousarray(x.T),
        "gwT": np.ascontiguousarray(gate_w.T),
        "gb": gate_bias.reshape(1, E),
        "ws1T": np.ascontiguousarray(ws1.T.astype(np.float16)),
        "ws3T": np.ascontiguousarray(ws3.T.astype(np.float16)),
        "ws2T": np.ascontiguousarray(ws2.T.astype(np.float16)),
    }
    common.update(_host_consts())
    in_maps = []
    for c in range(n_cores):
        e0 = (c * EL) % E
        sel = np.zeros((EL, E), np.float32)
        for le in range(EL):
            sel[le, e0 + le] = 1.0
        m = dict(common)
        m["esel"] = sel
        m["w1T"] = np.ascontiguousarray(
            w1[e0:e0 + EL].transpose(0, 2, 1).astype(np.float16))
        m["w3T"] = np.ascontiguousarray(
            w3[e0:e0 + EL].transpose(0, 2, 1).astype(np.float16))
        m["w2T"] = np.ascontiguousarray(
            w2[e0:e0 + EL].transpose(0, 2, 1).astype(np.float16))
        m["xTs"] = np.ascontiguousarray(x[c * TS:(c + 1) * TS].T.astype(np.float16))
        in_maps.append(m)
    return in_maps


def run_traced(inputs, trace=False, **kw):
    from concourse.bass_utils import run_bass_kernel_spmd

    nc = _get_nc(NCORES)
    in_maps = make_in_maps(inputs, NCORES)
    res = run_bass_kernel_spmd(nc, in_maps, core_ids=list(range(NCORES)),
                               trace=trace, **kw)
    slices = [res.results[c]["out"] for c in range(NCORES)]
    y = np.concatenate(slices, axis=0).reshape(*np.asarray(inputs["x"]).shape)
    return y.astype(np.float32), res


def kernel(**inputs) -> np.ndarray:
    return run_traced(inputs)[0]


# revision 24
# speedup vs baseline: 1.3577x; 1.0528x over previous
"""Trainium2 Bass kernel for nn_MoE_89498528514729 (moe_routing).

Expert-parallel sparse MoE across 8 NeuronCores:
  - every core gets the full x; routed experts are sharded 2-per-core
  - gate scores via fp32r matmul (full fp32 precision, 1 cycle/row)
  - group-limited top-4 routing computed token-major on DVE
  - per-expert token ranks via PE prefix-sum matmuls (triangular masks)
  - dispatch tables built with local_scatter; shard-merge via PE matmul
  - per-expert token gather via dma_gather (transposed, fp16)
  - SwiGLU expert FFN in fp16 (fp32 PSUM accumulation), capacity 576
  - weighted outputs scatter-added into a token-ordered partial-sum buffer
  - ReduceScatter combines partials across cores; each core finishes its
    256-token slice by adding the (token-sliced) shared expert output
Host side only shards/transposes/casts inputs and concatenates outputs.
"""

import numpy as np

import concourse.bass as bass
import concourse.mybir as mybir
import concourse.tile as tile
from concourse import bacc
from concourse.tile_rust import add_dep_helper

P = 128
T = 2048
D = 1024
II = 512
E = 16
EL = 2            # experts per core
NCORES = 8
TS = T // NCORES  # tokens per core output slice
C = 576           # per-expert compute capacity (actual max count 553)
CG = 640          # gather/scatter capacity (num_idxs must be 128-multiple)
CW = CG // 16     # wrapped index width
NT = T // P       # 16 token tiles
GC = 256          # gate chunk (tokens; fp32r needs >=256 for 1 cyc/row)
NGC = T // GC     # 4 chunks
TQ = 4            # token quarters for local_scatter layout
TC = T // TQ      # 512 tokens per quarter
BIG = 1.0e30
USE_SILU = True  # CoreSim lacks Silu; set False for CoreSim debugging

f32 = mybir.dt.float32
f32r = mybir.dt.float32r
f16 = mybir.dt.float16
i16 = mybir.dt.int16
i32 = mybir.dt.int32
Alu = mybir.AluOpType
Act = mybir.ActivationFunctionType


def build_kernel(n_cores: int = NCORES):
    nc = bacc.Bacc("TRN2", target_bir_lowering=False, debug=False, num_devices=n_cores)

    t_ = {}
    def inp(name, shape, dt):
        t_[name] = nc.dram_tensor(name, shape, dt, kind="ExternalInput")

    inp("x16", [T, D], f16)
    inp("xT32", [D, T], f32r)
    inp("gwT", [D, E], f32r)
    inp("gb", [1, E], f32)
    inp("esel", [EL, E], f32)
    inp("w1T", [EL, D, II], f16)
    inp("w3T", [EL, D, II], f16)
    inp("w2T", [EL, II, D], f16)
    inp("ws1T", [D, II], f16)
    inp("ws3T", [D, II], f16)
    inp("ws2T", [II, D], f16)
    inp("xTs", [D, TS], f16)
    inp("identf32", [E, E], f32)
    inp("identf16", [P, P], f16)
    inp("ltri", [P, P], f16)        # ltri[q, p] = q <= p
    inp("lse", [32, 32], f16)       # [(t' e'), (t e)] = (e'==e) & (t'<t)
    inp("selcnt", [32, EL], f16)    # [(t' e'), le] = (e'==le)
    inp("selmrg", [P, 32], f16)     # [(tq le s), (le' s')] = (le==le')&(s==s')
    inp("selrep", [EL, 32, P], f16)  # [le][(le' s), p] = (le'==le)&(s==p%16)
    inp("tok16", [P, TC], i16)      # tq(p)*TC + f + 1
    inp("sub16", [P, 1], f32)       # p % 16
    t_["out"] = nc.dram_tensor("out", [TS, D], f32, kind="ExternalOutput")

    with tile.TileContext(nc) as tc:
        _body(nc, tc, n_cores, t_)
    nc.compile()
    return nc


def _body(nc, tc, n_cores, t_):
    x16, xT32, gwT, gb, esel = t_["x16"], t_["xT32"], t_["gwT"], t_["gb"], t_["esel"]
    w1T, w3T, w2T = t_["w1T"], t_["w3T"], t_["w2T"]
    ws1T, ws3T, ws2T, xTs, out = t_["ws1T"], t_["ws3T"], t_["ws2T"], t_["xTs"], t_["out"]

    import contextlib
    ctx = contextlib.ExitStack()
    with ctx:
        const = ctx.enter_context(tc.tile_pool(name="const", bufs=1))
        wpool = ctx.enter_context(tc.tile_pool(name="wpool", bufs=1))
        gpool = ctx.enter_context(tc.tile_pool(name="gpool", bufs=1))
        spool = ctx.enter_context(tc.tile_pool(name="spool", bufs=2))
        xcp = ctx.enter_context(tc.tile_pool(name="xcp", bufs=2))
        xpool = ctx.enter_context(tc.tile_pool(name="xpool", bufs=2))
        hpool = ctx.enter_context(tc.tile_pool(name="hpool", bufs=1))
        ypool = ctx.enter_context(tc.tile_pool(name="ypool", bufs=1))
        ps_t = ctx.enter_context(tc.tile_pool(name="ps_t", bufs=2, space="PSUM"))
        ps_h = ctx.enter_context(tc.tile_pool(name="ps_h", bufs=2, space="PSUM"))
        ps_y = ctx.enter_context(tc.tile_pool(name="ps_y", bufs=2, space="PSUM"))
        dram = ctx.enter_context(tc.tile_pool(name="dram", bufs=1, space="DRAM"))

        # ---------------- DRAM internals ----------------
        comb_dram = dram.tile([T, 64], f32)
        g2_dram = dram.tile([32, 3, P], i32)   # rows (e,t); planes m2, rmod, rdiv+1
        y_dram = dram.tile([T, D], f16)
        rs_out = dram.tile([TS, D], f16)

        # ---------------- constant loads (Act queue; tiny) ----------------
        identg = const.tile([E, E], f32)
        nc.scalar.dma_start(identg[:], t_["identf32"][:, :])
        gwT_sb = const.tile([P, D // P, E], f32r)
        nc.scalar.dma_start(gwT_sb[:], gwT.ap().rearrange("(ko p) e -> p ko e", p=P))
        ident16 = const.tile([P, P], f16)
        nc.scalar.dma_start(ident16[:], t_["identf16"][:, :])
        ltri_sb = const.tile([P, P], f16)
        nc.scalar.dma_start(ltri_sb[:], t_["ltri"][:, :])
        lse_sb = const.tile([32, 32], f16)
        nc.scalar.dma_start(lse_sb[:], t_["lse"][:, :])
        selcnt_sb = const.tile([32, EL], f16)
        nc.scalar.dma_start(selcnt_sb[:], t_["selcnt"][:, :])
        selmrg_sb = const.tile([P, 32], f16)
        nc.scalar.dma_start(selmrg_sb[:], t_["selmrg"][:, :])
        selrep_sb = const.tile([32, EL, P], f16)
        nc.scalar.dma_start(selrep_sb[:], t_["selrep"].ap().rearrange("e k p -> k e p"))
        tok16_sb = const.tile([P, TC], i16)
        nc.scalar.dma_start(tok16_sb[:], t_["tok16"][:, :])
        sub16_sb = const.tile([P, 1], f32)
        nc.scalar.dma_start(sub16_sb[:], t_["sub16"][:, :])
        bias_sb = const.tile([P, E], f32)
        nc.scalar.dma_start(bias_sb[:], gb[0:1, :].to_broadcast([P, E]))
        esel_sb = const.tile([P, EL, E], f32)
        nc.scalar.dma_start(esel_sb[:], esel[None, :, :].to_broadcast([P, EL, E]))

        # zero tile for y_dram init (DVE, early)
        zero_sb = const.tile([P, D], f16)
        nc.vector.memset(zero_sb[:], 0.0)

        # ---------------- gate: scores chunks + transpose to token-major -----
        scores_all = gpool.tile([P, NT, E], f32)
        chunk_dmas = []
        for j in range(NGC):
            xg = xcp.tile([P, D // P, GC], f32r, tag="xgc")
            cdma = nc.sync.dma_start(
                xg[:], xT32.ap().rearrange("(ko p) t -> p ko t", p=P)[:, :, j * GC:(j + 1) * GC]
            )
            chunk_dmas.append(cdma)
            ps = ps_y.tile([P, GC], f32, tag="py")
            for k in range(D // P):
                nc.tensor.matmul(ps[:E, :],
                                 gwT_sb[:, k, :],
                                 xg[:, k, :],
                                 start=(k == 0), stop=(k == D // P - 1))
            sc = spool.tile([E, GC], f32, tag="scc")
            nc.scalar.activation(sc[:], ps[:E, :], Act.Sigmoid)
            for tt in range(GC // P):
                pst = ps_t.tile([P, E], f32, tag="tr")
                nc.tensor.transpose(pst[:], sc[:, tt * P:(tt + 1) * P], identg[:])
                nc.vector.tensor_copy(scores_all[:, j * (GC // P) + tt, :], pst[:])

        # bulk loads, fenced behind the gate-chunk DMAs so the serial DMA
        # device serves the gate (critical path) first
        fence7 = chunk_dmas[NGC - 2].ins
        def fenced_load(dst, src, fence):
            d = nc.sync.dma_start(dst, src)
            add_dep_helper(d.ins, fence, reason="DMA priority fence")
            return d
        ws1_sb = wpool.tile([P, D // P, II], f16, tag="ws1")
        fenced_load(ws1_sb[:], ws1T.ap().rearrange("(ko p) i -> p ko i", p=P), fence7)
        ws3_sb = wpool.tile([P, D // P, II], f16, tag="ws3")
        fenced_load(ws3_sb[:], ws3T.ap().rearrange("(ko p) i -> p ko i", p=P), fence7)
        xTs_sb = wpool.tile([P, D // P, TS], f16, tag="xTs")
        fenced_load(xTs_sb[:], xTs.ap().rearrange("(ko p) t -> p ko t", p=P), fence7)
        ws2_sb = wpool.tile([P, II // P, D], f16, tag="ws2")
        w1_sb = [wpool.tile([P, D // P, II], f16, tag=f"w1_{e}", name=f"w1_{e}")
                 for e in range(EL)]
        w3_sb = [wpool.tile([P, D // P, II], f16, tag=f"w3_{e}", name=f"w3_{e}")
                 for e in range(EL)]
        w2_sb = [wpool.tile([P, II // P, D], f16, tag=f"w2_{e}", name=f"w2_{e}")
                 for e in range(EL)]
        # w1/w3 for expert 0 and ws2 are loaded after the rank replication
        # DMA (they are needed only once the first gather completes)

        # ---------------- routing: group-limited top-4, token-major ----------
        # processed in quarters (4 token tiles each) so early quarters overlap
        # later gate-chunk DMAs; thresholds via the DVE sort-8 instruction
        mask4 = gpool.tile([P, NT, E], f32)
        comb = gpool.tile([P, NT, E], f32)
        comb2 = gpool.tile([P, NT, EL], f32)
        m01 = gpool.tile([P, NT, EL], f32)
        NQ = 4
        QW = NT // NQ
        v = nc.vector
        for q in range(NQ):
            a, b = q * QW, (q + 1) * QW
            w = b - a
            s_b = gpool.tile([P, QW, E], f32, tag="s_b", name="s_b")
            v.tensor_tensor(s_b[:], scores_all[:, a:b, :],
                            bias_sb[:, None, :].to_broadcast([P, w, E]), Alu.add)
            gs = gpool.tile([P, QW, 8], f32, tag="gs", name="gs")
            v.memset(gs[:, :, 4:], -BIG)
            v.tensor_reduce(gs[:, :, 0:4], s_b[:].rearrange("p a (g q) -> p a g q", q=4),
                            axis=mybir.AxisListType.X, op=Alu.max)
            g8 = gpool.tile([P, QW, 8], f32, tag="g8", name="g8")
            for t in range(QW):
                v.max(g8[:, t, :], gs[:, t, :])
            keep = gpool.tile([P, QW, 4], f32, tag="keep", name="keep")
            v.tensor_tensor(keep[:], gs[:, :, 0:4], g8[:, :, 1:2].to_broadcast([P, w, 4]),
                            Alu.is_ge)
            keepx = gpool.tile([P, QW, E], f32, tag="kx", name="kx")
            v.tensor_copy(keepx[:].rearrange("p a (g q) -> p a g q", q=4),
                          keep[:, :, :, None].to_broadcast([P, w, 4, 4]))
            # sm = keep ? s : -BIG  ==  keepx*s + (keepx - 1)*BIG
            sm_ = gpool.tile([P, QW, E], f32, tag="sm", name="sm")
            v.tensor_scalar(sm_[:], keepx[:], BIG, BIG, op0=Alu.mult, op1=Alu.subtract)
            kxs = gpool.tile([P, QW, E], f32, tag="kxs", name="kxs")
            v.tensor_tensor(kxs[:], s_b[:], keepx[:], Alu.mult)
            v.tensor_tensor(sm_[:], sm_[:], kxs[:], Alu.add)
            s8 = gpool.tile([P, QW, 8], f32, tag="s8", name="s8")
            for t in range(QW):
                v.max(s8[:, t, :], sm_[:, t, :])
            v.tensor_tensor(mask4[:, a:b, :], sm_[:],
                            s8[:, :, 3:4].to_broadcast([P, w, E]), Alu.is_ge)
            v.tensor_tensor(comb[:, a:b, :], mask4[:, a:b, :],
                            scores_all[:, a:b, :], Alu.mult)
            # local-expert combine weights + masks for this quarter
            for le in range(EL):
                tmp = gpool.tile([P, QW, E], f32, tag="seltmp", name="seltmp")
                sel = esel_sb[:, le, None, :].to_broadcast([P, w, E])
                v.tensor_tensor(tmp[:], comb[:, a:b, :], sel, Alu.mult)
                v.tensor_reduce(comb2[:, a:b, le], tmp[:], axis=mybir.AxisListType.X,
                                op=Alu.add)
                v.tensor_tensor(tmp[:], mask4[:, a:b, :], sel, Alu.mult)
                v.tensor_reduce(m01[:, a:b, le], tmp[:], axis=mybir.AxisListType.X,
                                op=Alu.add)

        # comb_dram rows (64-wide, cols 0:EL used); DMA is issued inside the
        # expert loop, fenced behind the first token gather
        cd = gpool.tile([P, NT, 64], f32)
        nc.vector.memset(cd[:, :, EL:], 0.0)
        nc.vector.tensor_copy(cd[:, :, 0:EL], comb2[:])

        # ---------------- ranks via PE prefix-sum matmuls --------------------
        m01f = gpool.tile([P, NT * EL], f16)
        nc.vector.tensor_copy(m01f[:].rearrange("p (e t) -> p t e", e=EL), m01[:])
        ps_incl = ps_t.tile([P, NT * EL], f32, tag="tr")
        nc.tensor.matmul(ps_incl[:], ltri_sb[:], m01f[:], start=True, stop=True)
        inclf = gpool.tile([P, NT * EL], f16)
        nc.vector.tensor_copy(inclf[:], ps_incl[:])
        # transpose incl and m01 to (tl, le)-major
        ps_iT = ps_t.tile([32, P], f16, tag="tr")
        nc.tensor.transpose(ps_iT[:], inclf[:], ident16[:])
        ps_mT = ps_t.tile([32, P], f16, tag="tr")
        nc.tensor.transpose(ps_mT[:], m01f[:], ident16[:])
        mgm = gpool.tile([32, P], f32)
        nc.vector.tensor_copy(mgm[:], ps_mT[:])
        mgr = gpool.tile([32, P], f32)
        nc.vector.tensor_copy(mgr[:], ps_iT[:])
        lastc = gpool.tile([32, 1], f16)
        nc.vector.tensor_copy(lastc[:], mgr[:, P - 1:P])
        ps_off = ps_t.tile([32, 1], f32, tag="tr")
        nc.tensor.matmul(ps_off[:], lse_sb[:], lastc[:], start=True, stop=True)
        off_sb = gpool.tile([32, 1], f32)
        nc.vector.tensor_copy(off_sb[:], ps_off[:])
        nc.vector.tensor_scalar(mgr[:], mgr[:], off_sb[:, 0:1], None, op0=Alu.add)
        ps_cnt = ps_t.tile([EL, 1], f32, tag="tr")
        nc.tensor.matmul(ps_cnt[:], selcnt_sb[:], lastc[:], start=True, stop=True)
        cnt_i = gpool.tile([EL, 1], i32)
        nc.vector.tensor_copy(cnt_i[:], ps_cnt[:])
        cnt_regs = []
        for e in range(EL):
            r = nc.alloc_register(mybir.EngineType.Pool, f"cnt{e}")
            nc.gpsimd.reg_load(r, cnt_i[e:e + 1, 0:1])
            cnt_regs.append(r)

        # small-side slot arithmetic on [32, P]: planes m2 = m & (r//16 < CW),
        # rmod = r % 16, rdivp1 = r//16 + 1 (r = exclusive rank)
        mga = gpool.tile([32, 3, P], i32)
        ri_s = gpool.tile([32, P], i32)
        nc.vector.tensor_copy(ri_s[:], mgr[:])
        mi_s = gpool.tile([32, P], i32)
        nc.vector.tensor_copy(mi_s[:], mgm[:])
        nc.vector.tensor_tensor(ri_s[:], ri_s[:], mi_s[:], Alu.subtract)
        nc.vector.tensor_scalar(mga[:, 1, :], ri_s[:], 15, None, op0=Alu.bitwise_and)
        rdiv_s = gpool.tile([32, P], i32)
        nc.vector.tensor_scalar(rdiv_s[:], ri_s[:], 4, None,
                                op0=Alu.logical_shift_right)
        gd_s = gpool.tile([32, P], i32)
        nc.vector.tensor_scalar(gd_s[:], rdiv_s[:], CW, None, op0=Alu.is_lt)
        nc.vector.tensor_tensor(mga[:, 0, :], mi_s[:], gd_s[:], Alu.bitwise_and)
        nc.vector.tensor_scalar(mga[:, 2, :], rdiv_s[:], 1, None, op0=Alu.add)

        # one bounce to DRAM; one broadcast reload to (le, tq, s) partitions
        nc.scalar.dma_start(g2_dram[:], mga[:])
        mrep3 = gpool.tile([P, TQ, 3, P], i32)
        mrep_dma = nc.scalar.dma_start(
            mrep3[:].rearrange("pp fl pl p -> pp (fl pl p)"),
            g2_dram[:].rearrange("(g fl) pl p -> g (fl pl p)", fl=TQ)
            [:, None, :].to_broadcast([EL * TQ, 16, TQ * 3 * P]))

        for dst, srcap in (
            (w1_sb[0][:], w1T[0].rearrange("(ko p) i -> p ko i", p=P)),
            (w3_sb[0][:], w3T[0].rearrange("(ko p) i -> p ko i", p=P)),
            (ws2_sb[:], ws2T.ap().rearrange("(ko p) d -> p ko d", p=P)),
        ):
            d = nc.sync.dma_start(dst, srcap)
            add_dep_helper(d.ins, mrep_dma.ins, reason="DMA priority fence")

        # slot indices: partition p=(le,tq,s); token f=(fl,p2) of quarter tq
        c1 = gpool.tile([P, TQ, P], i32, tag="c1")
        nc.vector.tensor_scalar(c1[:], mrep3[:, :, 1, :], sub16_sb[:, 0:1], None,
                                op0=Alu.is_equal)
        nc.vector.tensor_tensor(c1[:], c1[:], mrep3[:, :, 0, :], Alu.bitwise_and)
        nc.vector.tensor_tensor(c1[:], c1[:], mrep3[:, :, 2, :], Alu.mult)
        nc.vector.tensor_scalar(c1[:], c1[:], 1, None, op0=Alu.subtract)
        idx16 = gpool.tile([P, TC], i16)
        nc.vector.tensor_copy(idx16[:].rearrange("pp (fl p) -> pp fl p", fl=TQ), c1[:])
        gth4 = gpool.tile([P, CW], i16)
        nc.gpsimd.local_scatter(gth4[:], tok16_sb[:], idx16[:],
                                channels=P, num_elems=CW, num_idxs=TC)
        # merge the 4 token-quarter shards via PE, then replicate to 128 parts
        gthf = gpool.tile([P, CW], f16)
        nc.vector.tensor_copy(gthf[:], gth4[:])
        ps_mrg = ps_t.tile([32, CW], f32, tag="tr")
        nc.tensor.matmul(ps_mrg[:], selmrg_sb[:], gthf[:], start=True, stop=True)
        mrg_sb = gpool.tile([32, CW], f16)
        nc.vector.tensor_copy(mrg_sb[:], ps_mrg[:])
        gthx = []
        for e in range(EL):
            ps_rep = ps_t.tile([P, CW], f32, tag="tr")
            nc.tensor.matmul(ps_rep[:], selrep_sb[:, e, :], mrg_sb[:],
                             start=True, stop=True)
            g = gpool.tile([P, CW], i16, tag=f"gthx{e}")
            nc.vector.tensor_scalar(g[:], ps_rep[:], 1, None, op0=Alu.subtract)
            gthx.append(g)

        # ---------------- shared expert (h stage; z stage is emitted later) --
        hsT = gpool.tile([P, II // P, TS], f16, tag="hsT")
        for ic in range(II // P):
            p1 = ps_h.tile([P, TS], f32, tag="p1")
            p3 = ps_h.tile([P, TS], f32, tag="p3")
            for k in range(D // P):
                nc.tensor.matmul(p1[:], ws1_sb[:, k, ic * P:(ic + 1) * P], xTs_sb[:, k, :],
                                 start=(k == 0), stop=(k == D // P - 1))
            for k in range(D // P):
                nc.tensor.matmul(p3[:], ws3_sb[:, k, ic * P:(ic + 1) * P], xTs_sb[:, k, :],
                                 start=(k == 0), stop=(k == D // P - 1))
            s1 = spool.tile([P, TS], f32, tag="sh_s1")
            if USE_SILU:
                nc.scalar.activation(s1[:], p1[:], Act.Silu)
            else:
                nc.scalar.activation(s1[:], p1[:], Act.Sigmoid)
                nc.vector.tensor_tensor(s1[:], s1[:], p1[:], Alu.mult)
            nc.vector.tensor_tensor(hsT[:, ic, :], s1[:], p3[:], Alu.mult)

        # ---------------- routed experts -------------------------------------
        for e in range(EL):
            xgT = xpool.tile([P, D // P, CG], f16, tag="xgT")
            # slots >= count are never written by the gather; zero them so the
            # tail transpose (a PE matmul) cannot be poisoned by NaN garbage
            nc.vector.memset(xgT[:, :, 512:], 0.0)
            gxg = nc.gpsimd.dma_gather(xgT[:], x16[:], gthx[e][:], num_idxs=CG,
                                       num_idxs_reg=cnt_regs[e], elem_size=D,
                                       transpose=True, queue_num=0)
            if e == 0:
                # non-critical loads fenced behind the first token gather;
                # the comb_dram write must be emitted before the comb gathers
                gfence = gxg.ins
                def fenced_load2(dst, srcap):
                    d = nc.scalar.dma_start(dst, srcap)
                    add_dep_helper(d.ins, gfence, reason="DMA priority fence")
                    return d
                fenced_load2(comb_dram[:].rearrange("(o p) d -> p o d", p=P), cd[:])
            combg = xpool.tile([P, CG // P, 64], f32, tag="combg")
            nc.gpsimd.dma_gather(combg[:], comb_dram[:], gthx[e][:], num_idxs=CG,
                                 num_idxs_reg=cnt_regs[e], elem_size=64,
                                 transpose=False, queue_num=0)
            if e == 0:
                fenced_load2(w1_sb[1][:], w1T[1].rearrange("(ko p) i -> p ko i", p=P))
                fenced_load2(w3_sb[1][:], w3T[1].rearrange("(ko p) i -> p ko i", p=P))
                fenced_load2(w2_sb[0][:], w2T[0].rearrange("(ko p) d -> p ko d", p=P))
                fenced_load2(w2_sb[1][:], w2T[1].rearrange("(ko p) d -> p ko d", p=P))
                for o in range(4):
                    fenced_load2(
                        y_dram[:].rearrange("(o p) d -> p o d", p=P)[:, o * 4:(o + 1) * 4, :],
                        zero_sb[:, None, :].to_broadcast([P, 4, D]),
                    )
            hT = hpool.tile([P, II // P, C], f16, tag="hT")
            for ic in range(II // P):
                p1 = ps_h.tile([P, 512], f32, tag="p1")
                p3 = ps_h.tile([P, 512], f32, tag="p3")
                for k in range(D // P):
                    nc.tensor.matmul(p1[:], w1_sb[e][:, k, ic * P:(ic + 1) * P],
                                     xgT[:, k, 0:512],
                                     start=(k == 0), stop=(k == D // P - 1))
                for k in range(D // P):
                    nc.tensor.matmul(p3[:], w3_sb[e][:, k, ic * P:(ic + 1) * P],
                                     xgT[:, k, 0:512],
                                     start=(k == 0), stop=(k == D // P - 1))
                s1 = hpool.tile([P, 512], f32, tag="e_s1")
                if USE_SILU:
                    nc.scalar.activation(s1[:], p1[:], Act.Silu)
                else:
                    nc.scalar.activation(s1[:], p1[:], Act.Sigmoid)
                    nc.vector.tensor_tensor(s1[:], s1[:], p1[:], Alu.mult)
                nc.vector.tensor_tensor(hT[:, ic, 0:512], s1[:], p3[:], Alu.mult)
            # 64-token tail computed token-major (full-width mms, fewer instrs)
            CT = C - 512
            pt1 = ps_h.tile([P, 512], f32, tag="p1")
            pt3 = ps_h.tile([P, 512], f32, tag="p3")
            for k in range(D // P):
                nc.tensor.matmul(pt1[:CT, :], xgT[:, k, 512:C],
                                 w1_sb[e][:, k, :],
                                 start=(k == 0), stop=(k == D // P - 1))
            for k in range(D // P):
                nc.tensor.matmul(pt3[:CT, :], xgT[:, k, 512:C],
                                 w3_sb[e][:, k, :],
                                 start=(k == 0), stop=(k == D // P - 1))
            st1 = hpool.tile([P, 512], f32, tag="e_s1")
            if USE_SILU:
                nc.scalar.activation(st1[:CT, :], pt1[:CT, :], Act.Silu)
            else:
                nc.scalar.activation(st1[:CT, :], pt1[:CT, :], Act.Sigmoid)
                nc.vector.tensor_tensor(st1[:CT, :], st1[:CT, :], pt1[:CT, :], Alu.mult)
            htail = hpool.tile([P, 512], f16, tag="htail")
            nc.vector.tensor_tensor(htail[:CT, :], st1[:CT, :], pt3[:CT, :], Alu.mult)
            for ic in range(II // P):
                ptt = ps_t.tile([P, CT], f16, tag="tr")
                nc.tensor.transpose(ptt[:], htail[:CT, ic * P:(ic + 1) * P], ident16[:CT, :CT])
                nc.vector.tensor_copy(hT[:, ic, 512:C], ptt[:])
            yg = ypool.tile([P, CG // P, D], f16, tag="yg")
            for c5 in range((C + P - 1) // P):
                pw = min(P, C - c5 * P)
                for dc in range(D // 512):
                    py = ps_y.tile([P, 512], f32, tag="py")
                    for ic in range(II // P):
                        nc.tensor.matmul(py[:pw, :], hT[:, ic, c5 * P:c5 * P + pw],
                                         w2_sb[e][:, ic, dc * 512:(dc + 1) * 512],
                                         start=(ic == 0), stop=(ic == II // P - 1))
                    nc.vector.tensor_scalar(yg[:pw, c5, dc * 512:(dc + 1) * 512],
                                            py[:pw, :], combg[:pw, c5, e:e + 1], None,
                                            op0=Alu.mult)
            nc.gpsimd.dma_scatter_add(y_dram[:], yg[:], gthx[e][:], num_idxs=CG,
                                      num_idxs_reg=cnt_regs[e], elem_size=D,
                                      queue_num=0)

        # ---------------- shared expert z stage (fills PE gap near gathers) --
        zsb = gpool.tile([P, TS // P, D], f16, tag="zsb")
        for t2 in range(TS // P):
            for dc in range(D // 512):
                pz = ps_y.tile([P, 512], f32, tag="py")
                for ic in range(II // P):
                    nc.tensor.matmul(pz[:], hsT[:, ic, t2 * P:(t2 + 1) * P],
                                     ws2_sb[:, ic, dc * 512:(dc + 1) * 512],
                                     start=(ic == 0), stop=(ic == II // P - 1))
                nc.vector.tensor_copy(zsb[:, t2, dc * 512:(dc + 1) * 512], pz[:])

        # ---------------- cross-core reduce + finish ----------------
        if n_cores > 1:
            nc.gpsimd.collective_compute(
                "ReduceScatter", Alu.add,
                replica_groups=[list(range(n_cores))],
                ins=[y_dram[:].opt()],
                outs=[rs_out[:].opt()],
            )
        rs_src = rs_out if n_cores > 1 else y_dram
        for t2 in range(TS // P):
            rs_sb = spool.tile([P, D], f16, tag="rs_sb")
            nc.sync.dma_start(rs_sb[:], rs_src[t2 * P:(t2 + 1) * P, :])
            fin = spool.tile([P, D], f32, tag="fin")
            nc.vector.tensor_tensor(fin[:], zsb[:, t2, :], rs_sb[:], Alu.add)
            nc.sync.dma_start(out[t2 * P:(t2 + 1) * P, :], fin[:])


_NC_CACHE = {}


def _get_nc(n_cores=NCORES):
    if n_cores not in _NC_CACHE:
        _NC_CACHE[n_cores] = build_kernel(n_cores)
    return _NC_CACHE[n_cores]


def _host_consts():
    p = np.arange(P)
    q = np.arange(P)
    consts = {}
    consts["identf32"] = np.eye(E, dtype=np.float32)
    consts["identf16"] = np.eye(P, dtype=np.float16)
    consts["ltri"] = (q[:, None] <= p[None, :]).astype(np.float16)
    # rows/cols indexed by (e, t): idx = e*NT + t
    t_of = np.arange(32) % NT
    e_of = np.arange(32) // NT
    consts["lse"] = ((e_of[:, None] == e_of[None, :]) &
                     (t_of[:, None] < t_of[None, :])).astype(np.float16)
    consts["selcnt"] = (e_of[:, None] == np.arange(EL)[None, :]).astype(np.float16)
    # partition p = (le, tq, s): le = p>>6, tq = (p>>4)&3, s = p&15
    tq_p = (p >> 4) & 3
    le_p = p >> 6
    s_p = p & 15
    # cols (le', s'): idx = le'*16 + s'
    le_c = np.arange(32) >> 4
    s_c = np.arange(32) & 15
    consts["selmrg"] = ((le_p[:, None] == le_c[None, :]) &
                        (s_p[:, None] == s_c[None, :])).astype(np.float16)
    selrep = np.zeros((EL, 32, P), np.float16)
    for e in range(EL):
        selrep[e] = ((le_c[:, None] == e) & (s_c[:, None] == (p[None, :] & 15)))
    consts["selrep"] = selrep
    consts["tok16"] = (tq_p[:, None] * TC + np.arange(TC)[None, :] + 1).astype(np.int16)
    consts["sub16"] = s_p[:, None].astype(np.float32)
    return consts


def make_in_maps(inputs, n_cores=NCORES):
    x = np.asarray(inputs["x"], np.float32).reshape(T, D)
    gate_w = np.asarray(inputs["gate_w"], np.float32)
    gate_bias = np.asarray(inputs["gate_bias"], np.float32)
    w1 = np.asarray(inputs["w1"], np.float32)
    w2 = np.asarray(inputs["w2"], np.float32)
    w3 = np.asarray(inputs["w3"], np.float32)
    ws1 = np.asarray(inputs["ws1"], np.float32)
    ws2 = np.asarray(inputs["ws2"], np.float32)
    ws3 = np.asarray(inputs["ws3"], np.float32)

    common = {
        "x16": x.astype(np.float16),
        "xT32": np.ascontiguousarray(x.T),
        "gwT": np.ascontiguousarray(gate_w.T),
        "gb": gate_bias.reshape(1, E),
        "ws1T": np.ascontiguousarray(ws1.T.astype(np.float16)),
        "ws3T": np.ascontiguousarray(ws3.T.astype(np.float16)),
        "ws2T": np.ascontiguousarray(ws2.T.astype(np.float16)),
    }
    common.update(_host_consts())
    in_maps = []
    for c in range(n_cores):
        e0 = (c * EL) % E
        sel = np.zeros((EL, E), np.float32)
        for le in range(EL):
            sel[le, e0 + le] = 1.0
        m = dict(common)
        m["esel"] = sel
        m["w1T"] = np.ascontiguousarray(
            w1[e0:e0 + EL].transpose(0, 2, 1).astype(np.float16))
        m["w3T"] = np.ascontiguousarray(
            w3[e0:e0 + EL].transpose(0, 2, 1).astype(np.float16))
        m["w2T"] = np.ascontiguousarray(
            w2[e0:e0 + EL].transpose(0, 2, 1).astype(np.float16))
        m["xTs"] = np.ascontiguousarray(x[c * TS:(c + 1) * TS].T.astype(np.float16))
        in_maps.append(m)
    return in_maps


def run_traced(inputs, trace=False, **kw):
    from concourse.bass_utils import run_bass_kernel_spmd

    nc = _get_nc(NCORES)
    in_maps = make_in_maps(inputs, NCORES)
    res = run_bass_kernel_spmd(nc, in_maps, core_ids=list(range(NCORES)),
                               trace=trace, **kw)
    slices = [res.results[c]["out"] for c in range(NCORES)]
    y = np.concatenate(slices, axis=0).reshape(*np.asarray(inputs["x"]).shape)
    return y.astype(np.float32), res


def kernel(**inputs) -> np.ndarray:
    return run_traced(inputs)[0]
